# revision 1
# baseline (speedup 1.0000x reference)
"""DeepNCM Trainium2 kernel: prototype scatter-mean update + negative squared
L2 distances, data-parallel over embedding rows across 8 NeuronCores.

Contract: kernel(**inputs) takes the FULL unsharded inputs
(embeddings [65536,512] f32, prototypes [1000,512] f32, counter [1000] f32,
y_true [65536] int64) and returns the FULL output [65536,1000] f32.

Per-core plan (N_loc = 8192 rows):
  Phase 1: segment sums via one-hot matmul (lhsT=emb tile, rhs=onehot tile)
           accumulated in PSUM over 64 k-tiles; counts via DVE accumulation
           of the one-hot tiles + a ones-vector matmul reduction; e_sq via
           ScalarE Square with free-dim accumulation.
  AllReduce of [sums ; counts] (513x1000 f32) across the 8 cores.
  Prototype update (scatter_mean + running mean + where) computed per-class
  on-device, replicated on every core.
  Phase 2: cross = emb @ (2*protos)^T via PE (emb transposed on the fly with
           PE transpose-mode), epilogue out = 2*cross - e_sq - p_sq fused
           into ScalarE (per-partition bias) + VectorE (p_sq broadcast row).

Matmul operands are bf16 (accumulation in fp32 PSUM); everything scale-
sensitive (e_sq, prototype math, epilogue) stays fp32.
"""

import os
import sys
from contextlib import ExitStack

for _p in ("/opt/trn_rl_repo", "/root/.axon_site/_ro/trn_rl_repo"):
    if os.path.isdir(_p):
        if _p not in sys.path:
            sys.path.insert(0, _p)
        break

import numpy as np

import concourse.bass as bass
import concourse.mybir as mybir
import concourse.tile as tile
from concourse.masks import make_identity
from concourse.bass_utils import run_bass_kernel_spmd

N, D, C = 65536, 512, 1000
W = 8                      # cores
NL = N // W                # rows per core
P = 128
KT = NL // P               # 64 row tiles per core
DC = D // P                # 4 contraction chunks
CH = ((0, 512), (512, 1000))   # free-dim halves of the class axis
F32 = mybir.dt.float32
BF16 = mybir.dt.bfloat16
ALU = mybir.AluOpType
ACTF = mybir.ActivationFunctionType

# Toggled by test.py for profiling runs.
PROFILE = False
TRACE_KWARGS = {}
LAST_RESULT = [None]

_built = [None]


def _split_waits(nc, cap=1):
    """Walrus in this container rejects >1 sync-wait per instruction.
    Move excess waits onto preceding same-engine NOPs (in-order engines,
    so semantics are preserved)."""
    n_new = 0
    for fn in nc.m.functions:
        for bb in fn.blocks:
            new_list = []
            for ins in bb.instructions:
                si = getattr(ins, "sync_info", None)
                if si is not None and si.on_wait and len(si.on_wait) > cap:
                    waits = list(si.on_wait)
                    keep, rest = waits[:cap], waits[cap:]
                    for i in range(0, len(rest), cap):
                        nop = mybir.InstNoOp(
                            name=f"I-waitsplit-{n_new}", ins=[], outs=[]
                        )
                        n_new += 1
                        nop.engine = ins.engine
                        nop.sync_info = mybir.SyncInfo(
                            on_wait=rest[i : i + cap], on_update=[]
                        )
                        new_list.append(nop)
                    si.on_wait = keep
                new_list.append(ins)
            bb.instructions = new_list
    return n_new


def _build(unroll=1):
    nc = bass.Bass()
    emb_ext = nc.declare_dram_parameter("emb", [NL, D], F32, isOutput=False)
    yf_ext = nc.declare_dram_parameter("yf", [P, KT], F32, isOutput=False)
    counter_ext = nc.declare_dram_parameter("counter", [C], F32, isOutput=False)
    p0_ext = nc.declare_dram_parameter("p0", [C, D], F32, isOutput=False)
    out_ext = nc.declare_dram_parameter("out", [NL, C], F32, isOutput=True)

    with tile.TileContext(nc) as tc, ExitStack() as es:
        cpool = es.enter_context(tc.tile_pool(name="const", bufs=1))
        bpool = es.enter_context(tc.tile_pool(name="bigs", bufs=1))
        rpool = es.enter_context(tc.tile_pool(name="rows", bufs=1))
        in_pool = es.enter_context(tc.tile_pool(name="inp", bufs=4))
        oh_pool = es.enter_context(tc.tile_pool(name="oh", bufs=3))
        sq_pool = es.enter_context(tc.tile_pool(name="sq", bufs=2))
        etb_pool = es.enter_context(tc.tile_pool(name="etb", bufs=3))
        out_pool = es.enter_context(tc.tile_pool(name="outp", bufs=8))
        tmpb_pool = es.enter_context(tc.tile_pool(name="tmpb", bufs=2))
        dram = es.enter_context(tc.tile_pool(name="dram", bufs=1, space="DRAM"))

        # ---- constants ----
        ident = cpool.tile([P, P], F32, name="ident")
        make_identity(nc, ident[:])
        iota = cpool.tile([P, C], F32, name="iota")
        nc.gpsimd.iota(
            iota[:], pattern=[[1, C]], base=0, channel_multiplier=0,
            allow_small_or_imprecise_dtypes=True,
        )
        ones_col = cpool.tile([P, 1], BF16, name="onesc")
        nc.vector.memset(ones_col[:], 1.0)
        ones_row = cpool.tile([1, P], BF16, name="onesr")
        nc.vector.memset(ones_row[:], 1.0)

        y_sb = cpool.tile([P, KT], F32, name="y")
        nc.sync.dma_start(y_sb[:], yf_ext[:])
        e_sq = cpool.tile([P, KT], F32, name="esq")
        counts_acc = cpool.tile([P, C], BF16, name="cacc")
        nc.vector.memset(counts_acc[:], 0.0)

        sums_sb = bpool.tile([P, DC * C], BF16, name="sums")
        p0T = bpool.tile([P, DC * C], F32, name="p0T")  # later holds protosT
        A_b = bpool.tile([P, C], F32, tag="Abt", name="Ab")
        B_b = bpool.tile([P, C], F32, tag="Bbt", name="Bb")
        embT_full = bpool.tile([P, KT * D], BF16, name="embTf")

        for it_ in range(unroll):

            with tc.tile_pool(name=f"ps_sums{it_}", bufs=1, space="PSUM") as ps_sums:
                s_ps = [
                    [ps_sums.tile([P, c1 - c0], F32, tag=f"s{dc}_{ci}",
                                  name=f"s{dc}_{ci}")
                     for ci, (c0, c1) in enumerate(CH)]
                    for dc in range(DC)
                ]
                for kt in range(KT):
                    et = in_pool.tile([P, D], F32, tag="et", name="et")
                    nc.sync.dma_start(et[:], emb_ext[kt * P : (kt + 1) * P, :])
                    scr = sq_pool.tile([P, D], F32, tag="scr", name="scr")
                    nc.scalar.activation(
                        scr[:], et[:], ACTF.Square,
                        accum_out=e_sq[:, kt : kt + 1],
                    )
                    etb = etb_pool.tile([P, D], BF16, tag="etb", name="etb")
                    nc.gpsimd.tensor_copy(out=etb[:], in_=et[:])
                    oh = oh_pool.tile([P, C], BF16, tag="oh", name="oh")
                    nc.vector.tensor_scalar(
                        oh[:], iota[:], y_sb[:, kt : kt + 1], None, ALU.is_equal
                    )
                    nc.vector.tensor_tensor(
                        out=counts_acc[:], in0=counts_acc[:], in1=oh[:], op=ALU.add
                    )
                    for dc in range(DC):
                        lhs = etb[:, dc * P : (dc + 1) * P]
                        for ci, (c0, c1) in enumerate(CH):
                            nc.tensor.matmul(
                                s_ps[dc][ci][:], lhs, oh[:, c0:c1],
                                start=(kt == 0), stop=(kt == KT - 1),
                            )

                # negate e_sq once (used as ScalarE bias in phase 2)
                nc.vector.tensor_scalar(e_sq[:], e_sq[:], -1.0, None, ALU.mult)

                # sums psum -> sbuf (split between ScalarE / VectorE)
                for dc in range(DC):
                    for ci, (c0, c1) in enumerate(CH):
                        dst = sums_sb[:, dc * C + c0 : dc * C + c1]
                        if (dc + ci) % 2 == 0:
                            nc.scalar.copy(dst, s_ps[dc][ci][:])
                        else:
                            nc.vector.tensor_copy(out=dst, in_=s_ps[dc][ci][:])

            # ---- mid-kernel psum work: counts reduce, p0 transpose, coeffs ----
            with tc.tile_pool(name=f"ps_mid{it_}", bufs=1, space="PSUM") as ps_mid:
                # counts: reduce over partitions with ones-vector matmul
                counts_row = rpool.tile([1, C], F32, name="counts")
                for ci, (c0, c1) in enumerate(CH):
                    cp = ps_mid.tile([1, c1 - c0], F32, tag=f"r{ci}",
                                     name=f"cnt{ci}")
                    nc.tensor.matmul(
                        cp[:], ones_col[:], counts_acc[:, c0:c1],
                        start=True, stop=True,
                    )
                    nc.scalar.copy(counts_row[:, c0:c1], cp[:])

                # ---- all-reduce #1: counts only (tiny, finishes fast) ----
                cc1_in = dram.tile([1, C], F32, tag=f"c1i{it_}", name="c1i")
                cc1_out = dram.tile([1, C], F32, tag=f"c1o{it_}", name="c1o",
                                    addr_space="Shared")
                nc.sync.dma_start(cc1_in[:], counts_row[:])
                nc.gpsimd.collective_compute(
                    "AllReduce", ALU.add,
                    replica_groups=[list(range(W))],
                    ins=[cc1_in.opt()], outs=[cc1_out.opt()],
                )
                nc.sync.dma_start(counts_row[:], cc1_out[:])

                # ---- all-reduce #2: sums (big; overlapped with coeff math,
                # p0 transposes and the phase-2 transpose pre-staging) ----
                cc_in = dram.tile([DC * P, C], BF16, tag=f"ccin{it_}", name="ccin")
                cc_out = dram.tile([DC * P, C], BF16, tag=f"ccout{it_}",
                                   name="ccout", addr_space="Shared")
                for dc in range(DC):
                    nc.sync.dma_start(
                        cc_in[dc * P : (dc + 1) * P, :],
                        sums_sb[:, dc * C : (dc + 1) * C],
                    )
                nc.gpsimd.collective_compute(
                    "AllReduce", ALU.add,
                    replica_groups=[list(range(W))],
                    ins=[cc_in.opt()], outs=[cc_out.opt()],
                )
                for dc in range(DC):
                    nc.sync.dma_start(
                        sums_sb[:, dc * C : (dc + 1) * C],
                        cc_out[dc * P : (dc + 1) * P, :],
                    )

                # ---- p0 load + transpose (overlaps the collectives) ----
                for ct in range(8):
                    ncp = min(P, C - ct * P)
                    pt = in_pool.tile([P, D], F32, tag="et", name="p0t")
                    nc.sync.dma_start(
                        pt[0:ncp, :], p0_ext[ct * P : ct * P + ncp, :]
                    )
                    tr = ps_mid.tile([P, DC * P], F32, tag="tr", bufs=3,
                                     name="tr")
                    for dc in range(DC):
                        nc.tensor.matmul(
                            tr[:, dc * P : dc * P + ncp],
                            pt[0:ncp, dc * P : (dc + 1) * P],
                            ident[0:ncp, 0:ncp],
                            is_transpose=True,
                            start=(dc == 0), stop=(dc == DC - 1),
                        )
                    # strided single copy: psum block dc -> p0T chunk dc
                    dst = p0T.rearrange("p (dc c) -> p dc c", dc=DC)[
                        :, :, ct * P : ct * P + ncp]
                    srcv = tr.rearrange("p (dc q) -> p dc q", dc=DC)[:, :, 0:ncp]
                    if ct % 2 == 0:
                        nc.scalar.copy(dst, srcv)
                    else:
                        nc.vector.tensor_copy(out=dst, in_=srcv)

                # ---- pre-stage ALL phase-2 emb transposes (hidden under CC) ----
                for nt in range(KT):
                    et = in_pool.tile([P, D], F32, tag="et", name="et")
                    nc.sync.dma_start(et[:], emb_ext[nt * P : (nt + 1) * P, :])
                    tr = ps_mid.tile([P, DC * P], F32, tag="tr", bufs=3,
                                     name="tr")
                    for dc in range(DC):
                        nc.tensor.matmul(
                            tr[:, dc * P : (dc + 1) * P],
                            et[:, dc * P : (dc + 1) * P],
                            ident[:],
                            is_transpose=True,
                            start=(dc == 0), stop=(dc == DC - 1),
                        )
                    dst = embT_full[:, nt * D : (nt + 1) * D]
                    if nt % 2 == 0:
                        nc.scalar.copy(dst, tr[:])
                    else:
                        nc.vector.tensor_copy(out=dst, in_=tr[:])

                counter_row = rpool.tile([1, C], F32, name="ctr")
                nc.sync.dma_start(counter_row[:], counter_ext[None, :])

                # ---- per-class prototype coefficients (needs counts AR only) ----
                # protos = where(counts>0,
                #                (counter*p0 + sums/max(counts,1)) / (counter+1),
                #                p0)
                #        = A*p0 + B*sums;  we build 2A and 2B so the matmul rhs
                # protos2 = 2*protosT folds the cross-term factor of 2.
                rep = rpool.tile([1, C], F32, name="rep")
                nc.vector.tensor_scalar(rep[:], counts_row[:], 0.0, None, ALU.is_gt)
                tmp1 = rpool.tile([1, C], F32, tag="t1", name="t1")
                tmp2 = rpool.tile([1, C], F32, tag="t2", name="t2")
                A_row = rpool.tile([1, C], BF16, name="A")
                B_row = rpool.tile([1, C], BF16, name="B")
                # rm = 1/max(counts,1)
                nc.vector.tensor_scalar(tmp1[:], counts_row[:], 1.0, None, ALU.max)
                nc.vector.reciprocal(tmp1[:], tmp1[:])
                # rt = 1/(counter+1)
                nc.vector.tensor_scalar(tmp2[:], counter_row[:], 1.0, None, ALU.add)
                nc.vector.reciprocal(tmp2[:], tmp2[:])
                # 2B = 2 * rep * rm * rt
                nc.vector.tensor_tensor(out=B_row[:], in0=tmp1[:], in1=tmp2[:],
                                        op=ALU.mult)
                nc.vector.tensor_tensor(out=B_row[:], in0=B_row[:], in1=rep[:],
                                        op=ALU.mult)
                nc.vector.tensor_scalar(B_row[:], B_row[:], 2.0, None, ALU.mult)
                # 2A = 2 * (1 + rep * (counter*rt - 1))
                nc.vector.tensor_tensor(out=A_row[:], in0=counter_row[:],
                                        in1=tmp2[:], op=ALU.mult)
                nc.vector.tensor_scalar(A_row[:], A_row[:], 1.0, None, ALU.subtract)
                nc.vector.tensor_tensor(out=A_row[:], in0=A_row[:], in1=rep[:],
                                        op=ALU.mult)
                nc.vector.tensor_scalar(A_row[:], A_row[:], 1.0, None, ALU.add)
                nc.vector.tensor_scalar(A_row[:], A_row[:], 2.0, None, ALU.mult)

                # broadcast 2A,2B down partitions via ones outer-product
                for row, dst_b in ((A_row, A_b), (B_row, B_b)):
                    for ci, (c0, c1) in enumerate(CH):
                        ob = ps_mid.tile([P, c1 - c0], F32, tag="ob", bufs=2,
                                         name="ob")
                        nc.tensor.matmul(
                            ob[:], ones_row[:], row[:, c0:c1],
                            start=True, stop=True,
                        )
                        nc.scalar.copy(dst_b[:, c0:c1], ob[:])

                # p0T *= 2A (can run during the sums all-reduce)
                for dc in range(DC):
                    sl = slice(dc * C, (dc + 1) * C)
                    nc.vector.tensor_tensor(out=p0T[:, sl], in0=p0T[:, sl],
                                            in1=A_b[:], op=ALU.mult)

                # protos2 = 2A*p0T + 2B*sums  (bf16, the phase-2 matmul rhs)
                # interleaved with p_sq = 0.25 * sum_d protos2^2 per chunk
                protos2 = bpool.tile([P, DC * C], BF16, tag="pr2", name="pr2")
                psq_ps = [ps_mid.tile([1, c1 - c0], F32, tag=f"r{ci}",
                                      name=f"psq{ci}")
                          for ci, (c0, c1) in enumerate(CH)]
                for dc in range(DC):
                    sl = slice(dc * C, (dc + 1) * C)
                    tb = tmpb_pool.tile([P, C], F32, tag="tb", name="tb")
                    nc.vector.tensor_tensor(out=tb[:], in0=sums_sb[:, sl],
                                            in1=B_b[:], op=ALU.mult)
                    nc.vector.tensor_tensor(out=protos2[:, sl], in0=p0T[:, sl],
                                            in1=tb[:], op=ALU.add)
                    tbq = tmpb_pool.tile([P, C], BF16, tag="tbq", name="tbq")
                    nc.vector.tensor_tensor(out=tbq[:], in0=protos2[:, sl],
                                            in1=protos2[:, sl], op=ALU.mult)
                    for ci, (c0, c1) in enumerate(CH):
                        nc.tensor.matmul(
                            psq_ps[ci][:], ones_col[:], tbq[:, c0:c1],
                            start=(dc == 0), stop=(dc == DC - 1),
                        )
                p_sq_row = rpool.tile([1, C], BF16, tag="psqr", name="psqr")
                for ci, (c0, c1) in enumerate(CH):
                    nc.scalar.copy(p_sq_row[:, c0:c1], psq_ps[ci][:])
                p_sq_b = bpool.tile([P, C], F32, tag="Abt", name="psqb")
                for ci, (c0, c1) in enumerate(CH):
                    ob = ps_mid.tile([P, c1 - c0], F32, tag="ob", bufs=2, name="ob")
                    nc.tensor.matmul(
                        ob[:], ones_row[:], p_sq_row[:, c0:c1],
                        start=True, stop=True,
                    )
                    nc.scalar.mul(p_sq_b[:, c0:c1], ob[:], 0.25)

            # ---- phase 2: out = 2*emb@protosT' - e_sq - p_sq ----
            with tc.tile_pool(name=f"ps_cr{it_}", bufs=4, space="PSUM") as ps_cr:
                for nt in range(KT):
                    ot = out_pool.tile([P, C], F32, tag="ot", name="ot")
                    for ci, (c0, c1) in enumerate(CH):
                        cr = ps_cr.tile([P, c1 - c0], F32, tag=f"cr{ci}",
                                        name=f"cr{ci}")
                        for dc in range(DC):
                            nc.tensor.matmul(
                                cr[:],
                                embT_full[:, nt * D + dc * P : nt * D + (dc + 1) * P],
                                protos2[:, dc * C + c0 : dc * C + c1],
                                start=(dc == 0), stop=(dc == DC - 1),
                            )
                        nc.scalar.activation(
                            ot[:, c0:c1], cr[:], ACTF.Identity,
                            bias=e_sq[:, nt : nt + 1], scale=1.0,
                        )
                    nc.vector.tensor_tensor(
                        out=ot[:], in0=ot[:], in1=p_sq_b[:], op=ALU.subtract
                    )
                    nc.sync.dma_start(out_ext[nt * P : (nt + 1) * P, :], ot[:])

    _split_waits(nc)
    return nc


def kernel(embeddings, prototypes, counter, y_true):
    embeddings = np.ascontiguousarray(np.asarray(embeddings, dtype=np.float32))
    prototypes = np.ascontiguousarray(np.asarray(prototypes, dtype=np.float32))
    counter_f = np.ascontiguousarray(np.asarray(counter, dtype=np.float32))
    y = np.asarray(y_true)

    if _built[0] is None:
        _built[0] = _build()
    nc = _built[0]

    in_maps = []
    for i in range(W):
        sl = slice(i * NL, (i + 1) * NL)
        y_loc = y[sl].astype(np.float32)
        # partition-major labels: yf[p, t] = y_loc[t*128 + p]
        yf = np.ascontiguousarray(y_loc.reshape(KT, P).T)
        in_maps.append(
            {
                "emb": embeddings[sl],
                "yf": yf,
                "counter": counter_f,
                "p0": prototypes,
            }
        )

    res = run_bass_kernel_spmd(
        nc, in_maps, list(range(W)), trace=PROFILE, **TRACE_KWARGS
    )
    LAST_RESULT[0] = res
    out = np.concatenate([res.results[i]["out"] for i in range(W)], axis=0)
    return out.astype(np.float32, copy=False)



# revision 31
# speedup vs baseline: 2.1756x; 2.1756x over previous
"""DeepNCM Trainium2 kernel: prototype scatter-mean update + negative squared
L2 distances, data-parallel over embedding rows across 8 NeuronCores.

Contract: kernel(**inputs) takes the FULL unsharded inputs
(embeddings [65536,512] f32, prototypes [1000,512] f32, counter [1000] f32,
y_true [65536] int64) and returns the FULL output [65536,1000] f32.

Per-core plan (NL = 8192 rows, fp8 DoubleRow matmuls throughout):
  Phase 1 (class-chunk major): sumsT[c, d] = oh^T @ emb_aug via DoubleRow fp8
     matmuls (two row-tiles contracted per instruction); emb_aug carries a
     ones column so per-class counts fall out of the same matmuls.
     One-hot tiles generated on DVE+Pool; e_sq accumulated from a bf16 copy
     of emb via DVE/Pool scalar_tensor_tensor (fp32 accumulator).
  ReduceScatter [1000,516] bf16 -> each rank owns 125 classes (sums+counts).
  Per-rank prototype update -> protos2T fp8 [125,512] + (-p_sq) col, then
  AllGather fp8 [1000,516]; PE transposes give protos2 [d, c] + (-p_sq) row.
  Phase 2: out = 2*emb@protosT - e_sq - p_sq: PSUM accumulates a K=1
     DoubleRow matmul seeding -p_sq, then 2 DoubleRow fp8 matmuls
     (embT pairs x protos2); epilogue copy adds -e_sq (per-partition bias)
     split across Act/DVE/Pool; output DMAs split across SP/Act/Pool queues.
"""

import os
import sys
from contextlib import ExitStack

for _p in ("/opt/trn_rl_repo", "/root/.axon_site/_ro/trn_rl_repo"):
    if os.path.isdir(_p):
        if _p not in sys.path:
            sys.path.insert(0, _p)
        break

import numpy as np
import ml_dtypes

import concourse.bass as bass
import concourse.mybir as mybir
import concourse.tile as tile
from concourse.masks import make_identity
from concourse.bass_utils import run_bass_kernel_spmd

N, D, C = 65536, 512, 1000
W = 8                      # cores
NL = N // W                # 8192 rows per core
P = 128
KT = NL // P               # 64 row tiles per core
PR = KT // 2               # 32 row-tile pairs (DoubleRow)
CP = 1024                  # class axis padded to 8 chunks of 128
CH = 128                   # classes per rank / class chunk (incl. padding)
FA = 520                   # emb_aug width: 512 emb + ones col + 7 pad
FU = 516                   # used width in collective buffers
F32 = mybir.dt.float32
BF16 = mybir.dt.bfloat16
F8 = mybir.dt.float8e4
ALU = mybir.AluOpType
ACTF = mybir.ActivationFunctionType
DRM = mybir.MatmulPerfMode.DoubleRow

# Toggled by test.py for profiling runs.
PROFILE = False
TRACE_KWARGS = {}
LAST_RESULT = [None]

_built = [None]


def _split_waits(nc, cap=1):
    """Walrus in this container rejects >1 sync-wait per instruction.
    Move excess waits onto preceding same-engine NOPs (in-order engines,
    so semantics are preserved)."""
    n_new = 0
    for fn in nc.m.functions:
        for bb in fn.blocks:
            new_list = []
            for ins in bb.instructions:
                si = getattr(ins, "sync_info", None)
                if si is not None and si.on_wait and len(si.on_wait) > cap:
                    waits = list(si.on_wait)
                    keep, rest = waits[:cap], waits[cap:]
                    for i in range(0, len(rest), cap):
                        nop = mybir.InstNoOp(
                            name=f"I-waitsplit-{n_new}", ins=[], outs=[]
                        )
                        n_new += 1
                        nop.engine = ins.engine
                        nop.sync_info = mybir.SyncInfo(
                            on_wait=rest[i : i + cap], on_update=[]
                        )
                        new_list.append(nop)
                    si.on_wait = keep
                new_list.append(ins)
            bb.instructions = new_list
    return n_new


def _build():
    nc = bass.Bass()
    ea_ext = nc.declare_dram_parameter("ea", [PR, P, 2 * FA], F8, isOutput=False)
    eb_ext = nc.declare_dram_parameter("eb", [PR, P, 2 * D], BF16, isOutput=False)
    et_ext = nc.declare_dram_parameter("et", [P, 4 * NL], F8, isOutput=False)
    yf_ext = nc.declare_dram_parameter("yf", [P, KT], F32, isOutput=False)
    ctr_ext = nc.declare_dram_parameter("ctr", [CH, 1], F32, isOutput=False)
    p0s_ext = nc.declare_dram_parameter("p0s", [CH, D], F32, isOutput=False)
    out_ext = nc.declare_dram_parameter("out", [NL, C], F32, isOutput=True)

    with tile.TileContext(nc) as tc, ExitStack() as es:
        cpool = es.enter_context(tc.tile_pool(name="const", bufs=1))
        bpool = es.enter_context(tc.tile_pool(name="bigs", bufs=1))
        bigp = es.enter_context(tc.tile_pool(name="bigp", bufs=1))
        ebp = es.enter_context(tc.tile_pool(name="ebp", bufs=20))
        sqp = es.enter_context(tc.tile_pool(name="sqp", bufs=2))
        rp = es.enter_context(tc.tile_pool(name="rp", bufs=1))
        otp = es.enter_context(tc.tile_pool(name="otp", bufs=5))
        dram = es.enter_context(tc.tile_pool(name="dram", bufs=1, space="DRAM"))

        # ---- constants ----
        iota = cpool.tile([P, CP], F32, name="iota")
        nc.gpsimd.iota(
            iota[:], pattern=[[1, CP]], base=0, channel_multiplier=0,
            allow_small_or_imprecise_dtypes=True,
        )
        identb = cpool.tile([P, P], BF16, name="identb")
        make_identity(nc, identb[:])
        ones1 = cpool.tile([1, P], BF16, name="ones1")
        nc.vector.memset(ones1[:], 1.0)

        y_sb = cpool.tile([P, KT], F32, name="y")
        nc.sync.dma_start(y_sb[:], yf_ext[:])
        ctr_sb = rp.tile([CH, 1], F32, name="ctr")
        nc.sync.dma_start(ctr_sb[:], ctr_ext[:])
        p0s_sb = cpool.tile([CH, D], F32, name="p0s")
        nc.sync.dma_start(p0s_sb[:], p0s_ext[:])
        e_sq = cpool.tile([P, KT], F32, name="esq")

        # ---- big resident buffers ----
        # ea_full (phase 1) and embT (phase 2) are never live at the same
        # time: share one pool slot (same tag) to free 32KB/partition.
        ea_full = bigp.tile([P, PR * 2 * FA], F8, tag="big", name="ea")
        eav = ea_full.rearrange("p (pr j f) -> p pr j f", pr=PR, j=2)
        oh_full = bpool.tile([P, KT * CP], F8, name="oh")
        ohv = oh_full.rearrange("p (pr j c) -> p pr j c", pr=PR, j=2)
        p2sb = bpool.tile([P, 4 * CP], F8, name="p2sb")
        p2v = p2sb.rearrange("p (dc c) -> p dc c", dc=4)  # c = CP
        psq2 = cpool.tile([1, CP], BF16, name="psq2")
        nc.vector.memset(psq2[:], 0.0)
        ss = bpool.tile([P, 8 * FU], BF16, name="ss")
        ssv = ss.rearrange("p (cc f) -> p cc f", cc=8)

        # collective DRAM buffers
        ccin = dram.tile([CP, FU], BF16, name="ccin")
        rsout = dram.tile([CH, FU], BF16, name="rsout")
        agin = dram.tile([CH, FU], F8, name="agin")
        agout = dram.tile([CP, FU], F8, name="agout", addr_space="Shared")

        # ---- phase 1: loads + one-hot ----
        eb_tiles = []
        for pr in range(PR):
            nc.sync.dma_start(
                eav[:, pr, :, :], ea_ext[pr]
            )
            for j in (0, 1):
                kt = 2 * pr + j
                dst = ohv[:, pr, j, :]
                # split one-hot generation DVE : Pool roughly 39:25
                eng = nc.vector if (kt % 16) < 10 else nc.gpsimd
                eng.tensor_scalar(dst, iota[:], y_sb[:, kt : kt + 1], None,
                                  ALU.is_equal)

        # ---- e_sq: eb loads on SP, Square+accumulate on Act (Act is
        # otherwise idle until phase 2; runs through the collectives) ----
        for pr in range(PR):
            ebt = ebp.tile([P, 2 * D], BF16, tag="eb", name="eb")
            nc.sync.dma_start(ebt[:], eb_ext[pr])
            eb_tiles.append(ebt)
            for j in (0, 1):
                kt = 2 * pr + j
                scr = sqp.tile([P, D], BF16, tag="scr", name="scr")
                nc.scalar.activation(
                    scr[:], ebt[:, j * D : (j + 1) * D], ACTF.Square,
                    accum_out=e_sq[:, kt : kt + 1],
                )
        # negate e_sq once (used as per-partition bias in phase 2)
        nc.scalar.mul(e_sq[:], e_sq[:], -1.0)

        # counter-only coefficient work hoisted ahead of the ReduceScatter
        rt2 = rp.tile([CH, 1], F32, name="rt2")
        nc.vector.tensor_scalar(rt2[:], ctr_sb[:], 1.0, None, ALU.add)
        nc.vector.reciprocal(rt2[:], rt2[:])
        A2p = rp.tile([CH, 1], F32, name="A2p")
        nc.vector.tensor_tensor(out=A2p[:], in0=ctr_sb[:], in1=rt2[:], op=ALU.mult)
        nc.vector.tensor_scalar(A2p[:], A2p[:], 1.0, None, ALU.subtract)
        nc.vector.tensor_scalar(rt2[:], rt2[:], 2.0, None, ALU.mult)
        ones_c = rp.tile([CH, 1], F32, name="onesc")
        nc.vector.memset(ones_c[:], 1.0)
        agin_sb = rp.tile([CH, FU], F8, name="aginsb")
        nc.vector.memset(agin_sb[:, 513:516], 0.0)

        # ---- phase 1: segment sums via DoubleRow fp8 ----
        # Two passes of 4 class-chunks (4x psA + 4x psB = 8 PSUM banks);
        # pass 1 is row-pair-major so it pipelines with one-hot generation.
        for half in range(2):
            with tc.tile_pool(name=f"ps_seg{half}", bufs=1, space="PSUM") as psg:
                ccs = range(4 * half, 4 * half + 4)
                psAs = {cc: psg.tile([CH, 512], F32, tag=f"psA{cc}",
                                     name=f"psA{cc}") for cc in ccs}
                psBs = {cc: psg.tile([CH, 4], F32, tag=f"psB{cc}",
                                     name=f"psB{cc}") for cc in ccs}
                for pr in range(PR):
                    for cc in ccs:
                        lhs = ohv[:, pr, :, cc * CH : (cc + 1) * CH]
                        nc.tensor.matmul(
                            psAs[cc][:], lhs, eav[:, pr, :, 0:512],
                            start=(pr == 0), stop=(pr == PR - 1), perf_mode=DRM,
                        )
                        nc.tensor.matmul(
                            psBs[cc][:], lhs, eav[:, pr, :, 512:516],
                            start=(pr == 0), stop=(pr == PR - 1), perf_mode=DRM,
                        )
                for cc in ccs:
                    # copies on DVE: Act is jammed by eb DMAs, Pool must stay
                    # clear so the ReduceScatter can start early
                    nc.vector.tensor_copy(out=ssv[:, cc, 0:512],
                                          in_=psAs[cc][:])
                    nc.vector.tensor_copy(out=ssv[:, cc, 512:516],
                                          in_=psBs[cc][:])
                    # ccin DMAs ride the Pool queue (SP is busy with eb loads;
                    # Pool is idle between one-hot gen and the ReduceScatter)
                    nc.gpsimd.dma_start(
                        ccin[cc * CH : (cc + 1) * CH, :], ssv[:, cc, :]
                    )


        # ---- ReduceScatter (sums+counts, bf16) ----
        nc.gpsimd.collective_compute(
            "ReduceScatter", ALU.add,
            replica_groups=[list(range(W))],
            ins=[ccin.opt()], outs=[rsout.opt()],
        )

        # ---- embT load into ea_full's slot (overlaps the ReduceScatter) ----
        embT = bigp.tile([P, PR * 2 * FA], F8, tag="big", name="embT")
        etv = embT.rearrange("p (q n) -> p q n", q=4)[:, :, 0:NL]
        for q in range(4):
            nc.sync.dma_start(etv[:, q, :], et_ext[:, q * NL : (q + 1) * NL])

        # ---- per-rank prototype update (125 classes) ----
        # B2 = 2*rep*rm*rt ; A2 = 2*(1 + rep*(ctr*rt - 1)); rt2=2rt and
        # A2p=ctr*rt-1 were precomputed before the ReduceScatter.
        shard = rp.tile([CH, FU], BF16, name="shard")
        nc.sync.dma_start(shard[:], rsout[:])
        counts = shard[:, 512:513]
        rm = rp.tile([CH, 1], F32, name="rm")
        nc.vector.tensor_scalar(rm[:], counts, 1.0, None, ALU.max)
        nc.vector.reciprocal(rm[:], rm[:])
        rep = rp.tile([CH, 1], F32, name="rep")
        nc.vector.tensor_scalar(rep[:], counts, 0.0, None, ALU.is_gt)
        B2 = rp.tile([CH, 1], F32, name="B2")
        nc.vector.tensor_tensor(out=B2[:], in0=rm[:], in1=rt2[:], op=ALU.mult)
        nc.vector.tensor_tensor(out=B2[:], in0=B2[:], in1=rep[:], op=ALU.mult)
        A2 = rp.tile([CH, 1], F32, name="A2")
        nc.vector.scalar_tensor_tensor(
            out=A2[:], in0=A2p[:], scalar=rep[:], in1=ones_c[:],
            op0=ALU.mult, op1=ALU.add,
        )
        nc.vector.tensor_scalar(A2[:], A2[:], 2.0, None, ALU.mult)

        tB = rp.tile([CH, D], F32, name="tB")
        nc.vector.tensor_scalar(tB[:], shard[:, 0:512], B2[:], None, ALU.mult)
        nc.vector.scalar_tensor_tensor(
            out=agin_sb[:, 0:512], in0=p0s_sb[:], scalar=A2[:], in1=tB[:],
            op0=ALU.mult, op1=ALU.add,
        )
        # -p_sq = -0.25 * sum_d protos2^2 (from the quantized fp8 values)
        scr2 = rp.tile([CH, D], BF16, name="scr2")
        npsq = rp.tile([CH, 1], F32, name="npsq")
        nc.vector.scalar_tensor_tensor(
            out=scr2[:], in0=agin_sb[:, 0:512], scalar=-0.25,
            in1=agin_sb[:, 0:512], op0=ALU.mult, op1=ALU.mult,
            accum_out=npsq[:],
        )
        nc.vector.tensor_copy(out=agin_sb[:, 512:513], in_=npsq[:])
        nc.sync.dma_start(agin[:], agin_sb[:])

        # ---- AllGather (protos2T + -p_sq, fp8) ----
        nc.gpsimd.collective_compute(
            "AllGather", ALU.bypass,
            replica_groups=[list(range(W))],
            ins=[agin.opt()], outs=[agout.opt()],
        )

        # ---- transpose protos2T -> protos2 [d, c]; extract -p_sq row ----
        # (fp8 PE transpose is rejected by walrus; cast to bf16 on the load
        #  via gpsimd DMA and transpose in bf16.)
        pt_sb = rp.tile([CH, 8 * FU], BF16, name="ptsb")
        ptv = pt_sb.rearrange("p (cc f) -> p cc f", cc=8)
        agov = agout.rearrange("(cc p) f -> p cc f", cc=8)
        for cc in range(8):
            nc.gpsimd.dma_start(ptv[:, cc, :], agov[:, cc, :])
        with tc.tile_pool(name="ps_tr", bufs=3, space="PSUM") as pst:
            for cc in range(8):
                tr = pst.tile([P, 4 * P], BF16, tag="tr", name="tr")
                trv = tr.rearrange("p (dc c) -> p dc c", dc=4)
                for dc in range(4):
                    nc.tensor.matmul(
                        trv[:, dc, :],
                        ptv[:, cc, dc * P : (dc + 1) * P],
                        identb[:],
                        is_transpose=True,
                        start=(dc == 0), stop=(dc == 3),
                    )
                trq = pst.tile([4, CH], BF16, tag="trq", name="trq")
                nc.tensor.matmul(
                    trq[:], ptv[:, cc, 512:516], identb[:],
                    is_transpose=True, start=True, stop=True,
                )
                if cc % 2 == 0:
                    nc.scalar.copy(p2v[:, :, cc * CH : (cc + 1) * CH],
                                   trv[:, :, :])
                    nc.scalar.copy(psq2[0:1, cc * CH : (cc + 1) * CH],
                                   trq[0:1, :])
                else:
                    nc.vector.tensor_copy(
                        out=p2v[:, :, cc * CH : (cc + 1) * CH], in_=trv[:, :, :]
                    )
                    nc.vector.tensor_copy(
                        out=psq2[0:1, cc * CH : (cc + 1) * CH], in_=trq[0:1, :]
                    )

        # ---- phase 2: out = 2*emb@protosT - e_sq - p_sq ----
        with tc.tile_pool(name="ps_cr", bufs=4, space="PSUM") as ps_cr:
            for nt in range(KT):
                ot = otp.tile([P, CP], F32, tag="ot", name="ot")
                for h in range(2):
                    c0 = 512 * h
                    cr = ps_cr.tile([P, 512], F32, tag=f"cr{h}", name=f"cr{h}")
                    nc.tensor.matmul(
                        cr[:], ones1[:], psq2[0:1, c0 : c0 + 512],
                        start=True, stop=False,
                    )
                    for q in range(2):
                        nc.tensor.matmul(
                            cr[:],
                            etv[:, 2 * q : 2 * q + 2, nt * P : (nt + 1) * P],
                            p2v[:, 2 * q : 2 * q + 2, c0 : c0 + 512],
                            start=False, stop=(q == 1), perf_mode=DRM,
                        )
                    # epilogue: add -e_sq while copying psum -> sbuf
                    # (gpsimd cannot access PSUM, so Act/DVE only)
                    if (2 * nt + h) % 2 == 0:
                        nc.scalar.activation(
                            ot[:, c0 : c0 + 512], cr[:], ACTF.Identity,
                            bias=e_sq[:, nt : nt + 1], scale=1.0,
                        )
                    else:
                        nc.vector.tensor_scalar(
                            ot[:, c0 : c0 + 512], cr[:], e_sq[:, nt : nt + 1],
                            None, ALU.add,
                        )
                # output DMA rotation, finely interleaved: SP 30, Pool 28, Act 6
                r = nt % 16
                if r in (0, 2, 4, 6, 8, 10, 12, 13):
                    eng = nc.sync
                elif r in (1, 3, 5, 7, 9, 11, 14):
                    eng = nc.gpsimd
                else:
                    eng = nc.scalar
                eng.dma_start(out_ext[nt * P : (nt + 1) * P, :], ot[:, 0:C])

    _split_waits(nc)
    return nc


def _prep_inputs(embeddings, prototypes, counter, y_true):
    """Host-side sharding + layout prep (no kernel math beyond dtype casts)."""
    emb = np.ascontiguousarray(np.asarray(embeddings, dtype=np.float32))
    p0 = np.ascontiguousarray(np.asarray(prototypes, dtype=np.float32))
    ctr = np.ascontiguousarray(np.asarray(counter, dtype=np.float32))
    y = np.asarray(y_true)

    f8 = ml_dtypes.float8_e4m3
    bf = ml_dtypes.bfloat16

    p0_pad = np.zeros((CP, D), dtype=np.float32)
    p0_pad[0:C] = p0
    ctr_pad = np.zeros((CP,), dtype=np.float32)
    ctr_pad[0:C] = ctr

    in_maps = []
    for i in range(W):
        sl = slice(i * NL, (i + 1) * NL)
        e_i = emb[sl]                                   # [NL, D] f32
        # emb_aug fp8 pairs, partition-major: [PR, P, 2*FA]
        ea = np.zeros((NL, FA), dtype=f8)
        ea[:, 0:D] = e_i.astype(f8)
        ea[:, D] = 1.0
        ea_t = np.ascontiguousarray(
            ea.reshape(PR, 2, P, FA).transpose(0, 2, 1, 3).reshape(PR, P, 2 * FA)
        )
        # bf16 pairs for e_sq: [PR, P, 2*D]
        eb = e_i.astype(bf)
        eb_t = np.ascontiguousarray(
            eb.reshape(PR, 2, P, D).transpose(0, 2, 1, 3).reshape(PR, P, 2 * D)
        )
        # embT fp8: [P, 4*NL] with et[k, dc*NL + n] = emb[n, 128*dc + k]
        et = np.ascontiguousarray(
            e_i.astype(f8).T.reshape(4, P, NL).transpose(1, 0, 2).reshape(P, 4 * NL)
        )
        # labels, partition-major: yf[p, t] = y[t*128 + p]
        y_loc = y[sl].astype(np.float32)
        yf = np.ascontiguousarray(y_loc.reshape(KT, P).T)
        # per-rank class shard (class axis padded to CP)
        cs = slice(i * CH, (i + 1) * CH)
        in_maps.append(
            {
                "ea": ea_t,
                "eb": eb_t,
                "et": et,
                "yf": yf,
                "ctr": np.ascontiguousarray(ctr_pad[cs]).reshape(CH, 1),
                "p0s": np.ascontiguousarray(p0_pad[cs]),
            }
        )
    return in_maps


def kernel(embeddings, prototypes, counter, y_true):
    if _built[0] is None:
        _built[0] = _build()
    nc = _built[0]

    in_maps = _prep_inputs(embeddings, prototypes, counter, y_true)

    res = run_bass_kernel_spmd(
        nc, in_maps, list(range(W)), trace=PROFILE, **TRACE_KWARGS
    )
    LAST_RESULT[0] = res
    out = np.concatenate([res.results[i]["out"] for i in range(W)], axis=0)
    return out.astype(np.float32, copy=False)


# revision 37
# speedup vs baseline: 2.2340x; 1.0269x over previous
"""DeepNCM Trainium2 kernel: prototype scatter-mean update + negative squared
L2 distances, data-parallel over embedding rows across 8 NeuronCores.

Contract: kernel(**inputs) takes the FULL unsharded inputs
(embeddings [65536,512] f32, prototypes [1000,512] f32, counter [1000] f32,
y_true [65536] int64) and returns the FULL output [65536,1000] f32.

Per-core plan (NL = 8192 rows, fp8 DoubleRow matmuls throughout):
  Phase 1 (class-chunk major): sumsT[c, d] = oh^T @ emb_aug via DoubleRow fp8
     matmuls (two row-tiles contracted per instruction); emb_aug carries a
     ones column so per-class counts fall out of the same matmuls.
     One-hot tiles generated on DVE+Pool; e_sq accumulated from a bf16 copy
     of emb via DVE/Pool scalar_tensor_tensor (fp32 accumulator).
  ReduceScatter [1000,516] bf16 -> each rank owns 125 classes (sums+counts).
  Per-rank prototype update -> protos2T fp8 [125,512] + (-p_sq) col, then
  AllGather fp8 [1000,516]; PE transposes give protos2 [d, c] + (-p_sq) row.
  Phase 2: out = 2*emb@protosT - e_sq - p_sq: PSUM accumulates a K=1
     DoubleRow matmul seeding -p_sq, then 2 DoubleRow fp8 matmuls
     (embT pairs x protos2); epilogue copy adds -e_sq (per-partition bias)
     split across Act/DVE/Pool; output DMAs split across SP/Act/Pool queues.
"""

import os
import sys
from contextlib import ExitStack

for _p in ("/opt/trn_rl_repo", "/root/.axon_site/_ro/trn_rl_repo"):
    if os.path.isdir(_p):
        if _p not in sys.path:
            sys.path.insert(0, _p)
        break

import numpy as np
import ml_dtypes

import concourse.bass as bass
import concourse.mybir as mybir
import concourse.tile as tile
from concourse.masks import make_identity
from concourse.bass_utils import run_bass_kernel_spmd

N, D, C = 65536, 512, 1000
W = 8                      # cores
NL = N // W                # 8192 rows per core
P = 128
KT = NL // P               # 64 row tiles per core
PR = KT // 2               # 32 row-tile pairs (DoubleRow)
CP = 1024                  # class axis padded to 8 chunks of 128
CH = 128                   # classes per rank / class chunk (incl. padding)
FA = 520                   # emb_aug width: 512 emb + ones col + 7 pad
FU = 516                   # used width in collective buffers
F32 = mybir.dt.float32
BF16 = mybir.dt.bfloat16
F8 = mybir.dt.float8e4
ALU = mybir.AluOpType
ACTF = mybir.ActivationFunctionType
DRM = mybir.MatmulPerfMode.DoubleRow

# Toggled by test.py for profiling runs.
PROFILE = False
TRACE_KWARGS = {}
LAST_RESULT = [None]

_built = [None]


def _split_waits(nc, cap=1):
    """Walrus in this container rejects >1 sync-wait per instruction.
    Move excess waits onto preceding same-engine NOPs (in-order engines,
    so semantics are preserved)."""
    n_new = 0
    for fn in nc.m.functions:
        for bb in fn.blocks:
            new_list = []
            for ins in bb.instructions:
                si = getattr(ins, "sync_info", None)
                if si is not None and si.on_wait and len(si.on_wait) > cap:
                    waits = list(si.on_wait)
                    keep, rest = waits[:cap], waits[cap:]
                    for i in range(0, len(rest), cap):
                        nop = mybir.InstNoOp(
                            name=f"I-waitsplit-{n_new}", ins=[], outs=[]
                        )
                        n_new += 1
                        nop.engine = ins.engine
                        nop.sync_info = mybir.SyncInfo(
                            on_wait=rest[i : i + cap], on_update=[]
                        )
                        new_list.append(nop)
                    si.on_wait = keep
                new_list.append(ins)
            bb.instructions = new_list
    return n_new


def _build():
    nc = bass.Bass()
    ea_ext = nc.declare_dram_parameter("ea", [PR, P, 2 * FA], F8, isOutput=False)
    eb_ext = nc.declare_dram_parameter("eb", [PR, P, 2 * D], BF16, isOutput=False)
    et_ext = nc.declare_dram_parameter("et", [P, 4 * NL], F8, isOutput=False)
    yf_ext = nc.declare_dram_parameter("yf", [P, KT], F32, isOutput=False)
    ctr_ext = nc.declare_dram_parameter("ctr", [CH, 1], F32, isOutput=False)
    p0s_ext = nc.declare_dram_parameter("p0s", [CH, D], F32, isOutput=False)
    out_ext = nc.declare_dram_parameter("out", [NL, C], F32, isOutput=True)

    with tile.TileContext(nc) as tc, ExitStack() as es:
        cpool = es.enter_context(tc.tile_pool(name="const", bufs=1))
        bpool = es.enter_context(tc.tile_pool(name="bigs", bufs=1))
        bigp = es.enter_context(tc.tile_pool(name="bigp", bufs=1))
        ebp = es.enter_context(tc.tile_pool(name="ebp", bufs=20))
        sqp = es.enter_context(tc.tile_pool(name="sqp", bufs=2))
        rp = es.enter_context(tc.tile_pool(name="rp", bufs=1))
        otp = es.enter_context(tc.tile_pool(name="otp", bufs=6))
        dram = es.enter_context(tc.tile_pool(name="dram", bufs=1, space="DRAM"))

        # ---- constants ----
        iota = cpool.tile([P, CP], F32, name="iota")
        nc.gpsimd.iota(
            iota[:], pattern=[[1, CP]], base=0, channel_multiplier=0,
            allow_small_or_imprecise_dtypes=True,
        )
        identb = cpool.tile([P, P], BF16, name="identb")
        make_identity(nc, identb[:])
        ones2b = cpool.tile([2, 2 * P], F8, name="ones2b")
        nc.vector.memset(ones2b[:], 0.0)
        nc.vector.memset(ones2b[0:1, 0:P], 1.0)

        y_sb = cpool.tile([P, KT], F32, name="y")
        nc.sync.dma_start(y_sb[:], yf_ext[:])
        ctr_sb = rp.tile([CH, 1], F32, name="ctr")
        nc.sync.dma_start(ctr_sb[:], ctr_ext[:])
        p0s_sb = cpool.tile([CH, D], F32, name="p0s")
        nc.sync.dma_start(p0s_sb[:], p0s_ext[:])
        e_sq = cpool.tile([P, KT], F32, name="esq")

        # ---- big resident buffers ----
        # ea_full (phase 1) and embT (phase 2) are never live at the same
        # time: share one pool slot (same tag) to free 32KB/partition.
        ea_full = bigp.tile([P, PR * 2 * FA], F8, tag="big", name="ea")
        eav = ea_full.rearrange("p (pr j f) -> p pr j f", pr=PR, j=2)
        oh_full = bpool.tile([P, KT * CP], F8, name="oh")
        ohv = oh_full.rearrange("p (pr j c) -> p pr j c", pr=PR, j=2)
        p2sb = bpool.tile([P, 4 * CP], F8, name="p2sb")
        p2v = p2sb.rearrange("p (dc c) -> p dc c", dc=4)  # c = CP
        psq2b = cpool.tile([2, 2 * CP], F8, name="psq2b")
        nc.vector.memset(psq2b[:], 0.0)
        ss = bpool.tile([P, 8 * FU], BF16, name="ss")
        ssv = ss.rearrange("p (cc f) -> p cc f", cc=8)

        # collective DRAM buffers
        ccin = dram.tile([CP, FU], BF16, name="ccin")
        rsout = dram.tile([CH, FU], BF16, name="rsout")
        agin = dram.tile([513, P], F8, name="agin")
        agout = dram.tile([W * 513, P], F8, name="agout", addr_space="Shared")

        # ---- phase 1: loads + one-hot ----
        eb_tiles = []
        for pr in range(PR):
            nc.sync.dma_start(
                eav[:, pr, :, :], ea_ext[pr]
            )
            for j in (0, 1):
                kt = 2 * pr + j
                dst = ohv[:, pr, j, :]
                # split one-hot generation DVE : Pool roughly 39:25
                eng = nc.vector if (kt % 16) < 10 else nc.gpsimd
                eng.tensor_scalar(dst, iota[:], y_sb[:, kt : kt + 1], None,
                                  ALU.is_equal)

        # counter-only coefficient work hoisted ahead of the ReduceScatter
        rt2 = rp.tile([CH, 1], F32, name="rt2")
        nc.vector.tensor_scalar(rt2[:], ctr_sb[:], 1.0, None, ALU.add)
        nc.vector.reciprocal(rt2[:], rt2[:])
        A2p = rp.tile([CH, 1], F32, name="A2p")
        nc.vector.tensor_tensor(out=A2p[:], in0=ctr_sb[:], in1=rt2[:], op=ALU.mult)
        nc.vector.tensor_scalar(A2p[:], A2p[:], 1.0, None, ALU.subtract)
        nc.vector.tensor_scalar(A2p[:], A2p[:], 2.0, None, ALU.mult)
        nc.vector.tensor_scalar(rt2[:], rt2[:], 2.0, None, ALU.mult)
        twos_c = rp.tile([CH, 1], F32, name="twosc")
        nc.vector.memset(twos_c[:], 2.0)


        # ---- phase 1: segment sums via DoubleRow fp8 ----
        # Two passes of 4 class-chunks (4x psA + 4x psB = 8 PSUM banks);
        # pass 1 is row-pair-major so it pipelines with one-hot generation.
        def seg_copies(cc, psA, psB):
            # copies on DVE (Act runs the e_sq squares; Pool must stay
            # clear so the ReduceScatter can start early)
            nc.vector.tensor_copy(out=ssv[:, cc, 0:512], in_=psA[:])
            nc.vector.tensor_copy(out=ssv[:, cc, 512:516], in_=psB[:])
            # ccin DMAs ride the Pool queue (SP is busy with eb loads;
            # Pool is idle between one-hot gen and the ReduceScatter)
            nc.gpsimd.dma_start(ccin[cc * CH : (cc + 1) * CH, :], ssv[:, cc, :])

        # pass 1 (chunks 0-3): row-pair-major, pipelines with one-hot gen
        with tc.tile_pool(name="ps_seg0", bufs=1, space="PSUM") as psg:
            ccs = range(4)
            psAs = {cc: psg.tile([CH, 512], F32, tag=f"psA{cc}",
                                 name=f"psA{cc}") for cc in ccs}
            psBs = {cc: psg.tile([CH, 4], F32, tag=f"psB{cc}",
                                 name=f"psB{cc}") for cc in ccs}
            for pr in range(PR):
                for cc in ccs:
                    lhs = ohv[:, pr, :, cc * CH : (cc + 1) * CH]
                    nc.tensor.matmul(
                        psAs[cc][:], lhs, eav[:, pr, :, 0:512],
                        start=(pr == 0), stop=(pr == PR - 1), perf_mode=DRM,
                    )
                    nc.tensor.matmul(
                        psBs[cc][:], lhs, eav[:, pr, :, 512:516],
                        start=(pr == 0), stop=(pr == PR - 1), perf_mode=DRM,
                    )
            for cc in ccs:
                seg_copies(cc, psAs[cc], psBs[cc])
        # pass 2 (chunks 4-7): chunk-major so each chunk starts as soon as
        # one pass-1 bank pair frees, and its copy/DMA pipelines immediately
        with tc.tile_pool(name="ps_seg1", bufs=1, space="PSUM") as psg:
            for cc in range(4, 8):
                psA = psg.tile([CH, 512], F32, tag=f"psA{cc}", name=f"psA{cc}")
                psB = psg.tile([CH, 4], F32, tag=f"psB{cc}", name=f"psB{cc}")
                for pr in range(PR):
                    lhs = ohv[:, pr, :, cc * CH : (cc + 1) * CH]
                    nc.tensor.matmul(
                        psA[:], lhs, eav[:, pr, :, 0:512],
                        start=(pr == 0), stop=(pr == PR - 1), perf_mode=DRM,
                    )
                    nc.tensor.matmul(
                        psB[:], lhs, eav[:, pr, :, 512:516],
                        start=(pr == 0), stop=(pr == PR - 1), perf_mode=DRM,
                    )
                seg_copies(cc, psA, psB)


        # ---- ReduceScatter (sums+counts, bf16) ----
        nc.gpsimd.collective_compute(
            "ReduceScatter", ALU.add,
            replica_groups=[list(range(W))],
            ins=[ccin.opt()], outs=[rsout.opt()],
        )

        # ---- e_sq: eb loads on SP, Square+accumulate on Act (Act is
        # otherwise idle until phase 2; runs through the collectives) ----
        for pr in range(PR):
            ebt = ebp.tile([P, 2 * D], BF16, tag="eb", name="eb")
            nc.sync.dma_start(ebt[:], eb_ext[pr])
            eb_tiles.append(ebt)
            for j in (0, 1):
                kt = 2 * pr + j
                scr = sqp.tile([P, D], BF16, tag="scr", name="scr")
                nc.scalar.activation(
                    scr[:], ebt[:, j * D : (j + 1) * D], ACTF.Square,
                    accum_out=e_sq[:, kt : kt + 1],
                )
        # negate e_sq once (used as per-partition bias in phase 2)
        nc.scalar.mul(e_sq[:], e_sq[:], -1.0)

        # ---- embT load into ea_full's slot (overlaps the ReduceScatter) ----
        embT = bigp.tile([P, PR * 2 * FA], F8, tag="big", name="embT")
        etv = embT.rearrange("p (q n) -> p q n", q=4)[:, :, 0:NL]
        for q in range(4):
            nc.sync.dma_start(etv[:, q, :], et_ext[:, q * NL : (q + 1) * NL])

        # ---- per-rank prototype update (125 classes) ----
        # B2 = 2*rep*rm*rt ; A2 = 2*(1 + rep*(ctr*rt - 1)); rt2=2rt and
        # A2p=ctr*rt-1 were precomputed before the ReduceScatter.
        shard = rp.tile([CH, FU], BF16, name="shard")
        nc.sync.dma_start(shard[:], rsout[:])
        counts = shard[:, 512:513]
        rm = rp.tile([CH, 1], F32, name="rm")
        nc.vector.tensor_scalar(rm[:], counts, 1.0, None, ALU.max)
        nc.vector.reciprocal(rm[:], rm[:])
        rep = rp.tile([CH, 1], F32, name="rep")
        nc.vector.tensor_scalar(rep[:], counts, 0.0, None, ALU.is_gt)
        B2 = rp.tile([CH, 1], F32, name="B2")
        nc.vector.scalar_tensor_tensor(
            out=B2[:], in0=rm[:], scalar=rt2[:], in1=rep[:],
            op0=ALU.mult, op1=ALU.mult,
        )
        A2 = rp.tile([CH, 1], F32, name="A2")
        nc.vector.scalar_tensor_tensor(
            out=A2[:], in0=A2p[:], scalar=rep[:], in1=twos_c[:],
            op0=ALU.mult, op1=ALU.add,
        )

        tB = rp.tile([CH, D], F32, name="tB")
        nc.vector.tensor_scalar(tB[:], shard[:, 0:512], B2[:], None, ALU.mult)
        p2t_b = rp.tile([CH, D], BF16, name="p2tb")
        nc.vector.scalar_tensor_tensor(
            out=p2t_b[:], in0=p0s_sb[:], scalar=A2[:], in1=tB[:],
            op0=ALU.mult, op1=ALU.add,
        )
        # -p_sq = -0.25 * sum_d protos2^2
        scr2 = rp.tile([CH, D], BF16, name="scr2")
        npsq_b = rp.tile([CH, 1], BF16, name="npsqb")
        nc.vector.scalar_tensor_tensor(
            out=scr2[:], in0=p2t_b[:], scalar=-0.25, in1=p2t_b[:],
            op0=ALU.mult, op1=ALU.mult, accum_out=npsq_b[:],
        )

        # transpose this rank's protos2T to [d, c] BEFORE the AllGather so
        # no transpose work sits on the post-collective critical path
        agst = rp.tile([P, 4 * P], F8, name="agst")
        agsv = agst.rearrange("p (dc c) -> p dc c", dc=4)
        psq_st = rp.tile([1, P], F8, name="psqst")
        with tc.tile_pool(name="ps_tr", bufs=1, space="PSUM") as pst:
            t2 = pst.tile([P, 4 * P], BF16, tag="t2", name="t2")
            t2v = t2.rearrange("p (dc c) -> p dc c", dc=4)
            for dc in range(4):
                nc.tensor.matmul(
                    t2v[:, dc, :], p2t_b[:, dc * P : (dc + 1) * P], identb[:],
                    is_transpose=True, start=(dc == 0), stop=(dc == 3),
                )
            tq2 = pst.tile([1, P], BF16, tag="tq2", name="tq2")
            nc.tensor.matmul(tq2[:], npsq_b[:], identb[:],
                             is_transpose=True, start=True, stop=True)
            nc.vector.tensor_copy(out=agst[:], in_=t2[:])
            nc.scalar.copy(psq_st[:], tq2[:])
        # agin rows 0..511 = protos2 chunk [d, c]; row 512 = -p_sq row
        nc.sync.dma_start(
            agin[0:512, :].rearrange("(dc p) c -> p dc c", dc=4),
            agsv[:, :, :],
        )
        nc.scalar.dma_start(agin[512:513, :], psq_st[:])

        # ---- AllGather (protos2T + -p_sq, fp8) ----
        nc.gpsimd.collective_compute(
            "AllGather", ALU.bypass,
            replica_groups=[list(range(W))],
            ins=[agin.opt()], outs=[agout.opt()],
        )

        # ---- load gathered protos2 (already [d, c] per rank) + -p_sq row ----
        agov = agout.rearrange("(r q) c -> q r c", r=W)
        for dc in range(4):
            eng = nc.sync if dc < 2 else nc.scalar
            eng.dma_start(
                p2sb.rearrange("p (dc r c) -> p dc r c", dc=4, r=W)[:, dc, :, :],
                agov[dc * P : (dc + 1) * P, :, :],
            )
        nc.sync.dma_start(
            psq2b[0:1, 0:CP].rearrange("a (r c) -> a r c", r=W),
            agov[512:513, :, :],
        )

        # ---- phase 2: out = 2*emb@protosT - e_sq - p_sq ----
        with tc.tile_pool(name="ps_cr", bufs=4, space="PSUM") as ps_cr:
            for nt in range(KT):
                ot = otp.tile([P, CP], F32, tag="ot", name="ot")
                for h in range(2):
                    c0 = 512 * h
                    cr = ps_cr.tile([P, 512], F32, tag=f"cr{h}", name=f"cr{h}")
                    nc.tensor.matmul(
                        cr[:],
                        ones2b.rearrange("k (j m) -> k j m", j=2)[:, :, :],
                        psq2b.rearrange("k (j c) -> k j c", j=2)[:, :, c0 : c0 + 512],
                        start=True, stop=False, perf_mode=DRM,
                    )
                    for q in range(2):
                        nc.tensor.matmul(
                            cr[:],
                            etv[:, 2 * q : 2 * q + 2, nt * P : (nt + 1) * P],
                            p2v[:, 2 * q : 2 * q + 2, c0 : c0 + 512],
                            start=False, stop=(q == 1), perf_mode=DRM,
                        )
                    # epilogue: add -e_sq while copying psum -> sbuf
                    # (gpsimd cannot access PSUM, so Act/DVE only)
                    if (2 * nt + h) % 2 == 0:
                        nc.scalar.activation(
                            ot[:, c0 : c0 + 512], cr[:], ACTF.Identity,
                            bias=e_sq[:, nt : nt + 1], scale=1.0,
                        )
                    else:
                        nc.vector.tensor_scalar(
                            ot[:, c0 : c0 + 512], cr[:], e_sq[:, nt : nt + 1],
                            None, ALU.add,
                        )
                # output DMA rotation, finely interleaved: SP 30, Pool 28, Act 6
                r = nt % 16
                if r in (0, 2, 4, 6, 8, 10, 12, 13):
                    eng = nc.sync
                elif r in (1, 3, 5, 7, 9, 11, 14):
                    eng = nc.gpsimd
                else:
                    eng = nc.scalar
                eng.dma_start(out_ext[nt * P : (nt + 1) * P, :], ot[:, 0:C])

    _split_waits(nc)
    return nc


def _prep_inputs(embeddings, prototypes, counter, y_true):
    """Host-side sharding + layout prep (no kernel math beyond dtype casts)."""
    emb = np.ascontiguousarray(np.asarray(embeddings, dtype=np.float32))
    p0 = np.ascontiguousarray(np.asarray(prototypes, dtype=np.float32))
    ctr = np.ascontiguousarray(np.asarray(counter, dtype=np.float32))
    y = np.asarray(y_true)

    f8 = ml_dtypes.float8_e4m3
    bf = ml_dtypes.bfloat16

    p0_pad = np.zeros((CP, D), dtype=np.float32)
    p0_pad[0:C] = p0
    ctr_pad = np.zeros((CP,), dtype=np.float32)
    ctr_pad[0:C] = ctr

    in_maps = []
    for i in range(W):
        sl = slice(i * NL, (i + 1) * NL)
        e_i = emb[sl]                                   # [NL, D] f32
        # emb_aug fp8 pairs, partition-major: [PR, P, 2*FA]
        ea = np.zeros((NL, FA), dtype=f8)
        ea[:, 0:D] = e_i.astype(f8)
        ea[:, D] = 1.0
        ea_t = np.ascontiguousarray(
            ea.reshape(PR, 2, P, FA).transpose(0, 2, 1, 3).reshape(PR, P, 2 * FA)
        )
        # bf16 pairs for e_sq: [PR, P, 2*D]
        eb = e_i.astype(bf)
        eb_t = np.ascontiguousarray(
            eb.reshape(PR, 2, P, D).transpose(0, 2, 1, 3).reshape(PR, P, 2 * D)
        )
        # embT fp8: [P, 4*NL] with et[k, dc*NL + n] = emb[n, 128*dc + k]
        et = np.ascontiguousarray(
            e_i.astype(f8).T.reshape(4, P, NL).transpose(1, 0, 2).reshape(P, 4 * NL)
        )
        # labels, partition-major: yf[p, t] = y[t*128 + p]
        y_loc = y[sl].astype(np.float32)
        yf = np.ascontiguousarray(y_loc.reshape(KT, P).T)
        # per-rank class shard (class axis padded to CP)
        cs = slice(i * CH, (i + 1) * CH)
        in_maps.append(
            {
                "ea": ea_t,
                "eb": eb_t,
                "et": et,
                "yf": yf,
                "ctr": np.ascontiguousarray(ctr_pad[cs]).reshape(CH, 1),
                "p0s": np.ascontiguousarray(p0_pad[cs]),
            }
        )
    return in_maps


def kernel(embeddings, prototypes, counter, y_true):
    if _built[0] is None:
        _built[0] = _build()
    nc = _built[0]

    in_maps = _prep_inputs(embeddings, prototypes, counter, y_true)

    res = run_bass_kernel_spmd(
        nc, in_maps, list(range(W)), trace=PROFILE, **TRACE_KWARGS
    )
    LAST_RESULT[0] = res
    out = np.concatenate([res.results[i]["out"] for i in range(W)], axis=0)
    return out.astype(np.float32, copy=False)


# revision 45
# speedup vs baseline: 2.3013x; 1.0301x over previous
"""DeepNCM Trainium2 kernel: prototype scatter-mean update + negative squared
L2 distances, data-parallel over embedding rows across 8 NeuronCores.

Contract: kernel(**inputs) takes the FULL unsharded inputs
(embeddings [65536,512] f32, prototypes [1000,512] f32, counter [1000] f32,
y_true [65536] int64) and returns the FULL output [65536,1000] f32.

Per-core plan (NL = 8192 rows, fp8 DoubleRow matmuls throughout):
  Phase 1 (class-chunk major): sumsT[c, d] = oh^T @ emb_aug via DoubleRow fp8
     matmuls (two row-tiles contracted per instruction); emb_aug carries a
     ones column so per-class counts fall out of the same matmuls.
     One-hot tiles generated on DVE+Pool; e_sq accumulated from a bf16 copy
     of emb via DVE/Pool scalar_tensor_tensor (fp32 accumulator).
  ReduceScatter [1000,516] bf16 -> each rank owns 125 classes (sums+counts).
  Per-rank prototype update -> protos2T fp8 [125,512] + (-p_sq) col, then
  AllGather fp8 [1000,516]; PE transposes give protos2 [d, c] + (-p_sq) row.
  Phase 2: out = 2*emb@protosT - e_sq - p_sq: PSUM accumulates a K=1
     DoubleRow matmul seeding -p_sq, then 2 DoubleRow fp8 matmuls
     (embT pairs x protos2); epilogue copy adds -e_sq (per-partition bias)
     split across Act/DVE/Pool; output DMAs split across SP/Act/Pool queues.
"""

import os
import sys
from contextlib import ExitStack

for _p in ("/opt/trn_rl_repo", "/root/.axon_site/_ro/trn_rl_repo"):
    if os.path.isdir(_p):
        if _p not in sys.path:
            sys.path.insert(0, _p)
        break

import numpy as np
import ml_dtypes

import concourse.bass as bass
import concourse.mybir as mybir
import concourse.tile as tile
from concourse.masks import make_identity
from concourse.bass_utils import run_bass_kernel_spmd

N, D, C = 65536, 512, 1000
W = 8                      # cores
NL = N // W                # 8192 rows per core
P = 128
KT = NL // P               # 64 row tiles per core
PR = KT // 2               # 32 row-tile pairs (DoubleRow)
CP = 1024                  # class axis padded to 8 chunks of 128
CH = 128                   # classes per rank / class chunk (incl. padding)
FA = 520                   # emb_aug width: 512 emb + ones col + 7 pad
FU = 516                   # used width in collective buffers
F32 = mybir.dt.float32
BF16 = mybir.dt.bfloat16
F8 = mybir.dt.float8e4
ALU = mybir.AluOpType
ACTF = mybir.ActivationFunctionType
DRM = mybir.MatmulPerfMode.DoubleRow

# Toggled by test.py for profiling runs.
PROFILE = False
TRACE_KWARGS = {}
LAST_RESULT = [None]

_built = [None]


def _split_waits(nc, cap=1):
    """Walrus in this container rejects >1 sync-wait per instruction.
    Move excess waits onto preceding same-engine NOPs (in-order engines,
    so semantics are preserved)."""
    n_new = 0
    for fn in nc.m.functions:
        for bb in fn.blocks:
            new_list = []
            for ins in bb.instructions:
                si = getattr(ins, "sync_info", None)
                if si is not None and si.on_wait and len(si.on_wait) > cap:
                    waits = list(si.on_wait)
                    keep, rest = waits[:cap], waits[cap:]
                    for i in range(0, len(rest), cap):
                        nop = mybir.InstNoOp(
                            name=f"I-waitsplit-{n_new}", ins=[], outs=[]
                        )
                        n_new += 1
                        nop.engine = ins.engine
                        nop.sync_info = mybir.SyncInfo(
                            on_wait=rest[i : i + cap], on_update=[]
                        )
                        new_list.append(nop)
                    si.on_wait = keep
                new_list.append(ins)
            bb.instructions = new_list
    return n_new


def _build():
    nc = bass.Bass()
    ea_ext = nc.declare_dram_parameter("ea", [PR, P, 2 * FA], F8, isOutput=False)
    eb_ext = nc.declare_dram_parameter("eb", [PR, P, 2 * D], BF16, isOutput=False)
    et_ext = nc.declare_dram_parameter("et", [P, 4 * NL], F8, isOutput=False)
    yf_ext = nc.declare_dram_parameter("yf", [P, KT], F32, isOutput=False)
    ctr_ext = nc.declare_dram_parameter("ctr", [CH, 1], F32, isOutput=False)
    p0s_ext = nc.declare_dram_parameter("p0s", [CH, D], F32, isOutput=False)
    out_ext = nc.declare_dram_parameter("out", [NL, C], F32, isOutput=True)

    with tile.TileContext(nc) as tc, ExitStack() as es:
        cpool = es.enter_context(tc.tile_pool(name="const", bufs=1))
        bpool = es.enter_context(tc.tile_pool(name="bigs", bufs=1))
        bigp = es.enter_context(tc.tile_pool(name="bigp", bufs=1))
        ebp = es.enter_context(tc.tile_pool(name="ebp", bufs=20))
        sqp = es.enter_context(tc.tile_pool(name="sqp", bufs=2))
        rp = es.enter_context(tc.tile_pool(name="rp", bufs=1))
        otp = es.enter_context(tc.tile_pool(name="otp", bufs=6))
        dram = es.enter_context(tc.tile_pool(name="dram", bufs=1, space="DRAM"))

        # ---- constants ----
        iota = cpool.tile([P, CP], F32, name="iota")
        nc.gpsimd.iota(
            iota[:], pattern=[[1, CP]], base=0, channel_multiplier=0,
            allow_small_or_imprecise_dtypes=True,
        )
        identb = cpool.tile([P, P], BF16, name="identb")
        make_identity(nc, identb[:])
        ones2b = cpool.tile([2, 2 * P], F8, name="ones2b")
        nc.vector.memset(ones2b[:], 0.0)
        nc.vector.memset(ones2b[0:1, 0:P], 1.0)

        y_sb = cpool.tile([P, KT], F32, name="y")
        nc.sync.dma_start(y_sb[:], yf_ext[:])
        ctr_sb = rp.tile([CH, 1], F32, name="ctr")
        nc.sync.dma_start(ctr_sb[:], ctr_ext[:])
        p0s_sb = cpool.tile([CH, D], F32, name="p0s")
        nc.sync.dma_start(p0s_sb[:], p0s_ext[:])
        e_sq = cpool.tile([P, KT], F32, name="esq")

        # ---- big resident buffers ----
        # ea_full (phase 1) and embT (phase 2) are never live at the same
        # time: share one pool slot (same tag) to free 32KB/partition.
        ea_full = bigp.tile([P, PR * 2 * FA], F8, tag="big", name="ea")
        eav = ea_full.rearrange("p (pr j f) -> p pr j f", pr=PR, j=2)
        oh_full = bpool.tile([P, KT * CP], F8, name="oh")
        ohv = oh_full.rearrange("p (pr j c) -> p pr j c", pr=PR, j=2)
        p2sb = bpool.tile([P, 4 * CP], F8, name="p2sb")
        p2v = p2sb.rearrange("p (dc c) -> p dc c", dc=4)  # c = CP
        psq2b = cpool.tile([2, 2 * CP], F8, name="psq2b")
        nc.vector.memset(psq2b[:], 0.0)
        ss = bpool.tile([P, 8 * FU], BF16, name="ss")
        ssv = ss.rearrange("p (cc f) -> p cc f", cc=8)

        # collective DRAM buffers
        ccin = dram.tile([CP, FU], BF16, name="ccin")
        rsout = dram.tile([CH, FU], BF16, name="rsout")
        agin = dram.tile([513, P], F8, name="agin")
        agout = dram.tile([W * 513, P], F8, name="agout", addr_space="Shared")

        # ---- phase 1: loads + one-hot ----
        eb_tiles = []
        for pr in range(PR):
            nc.sync.dma_start(
                eav[:, pr, :, :], ea_ext[pr]
            )
            for j in (0, 1):
                kt = 2 * pr + j
                dst = ohv[:, pr, j, :]
                # split one-hot generation DVE : Pool roughly 39:25
                eng = nc.vector if (kt % 16) < 10 else nc.gpsimd
                eng.tensor_scalar(dst, iota[:], y_sb[:, kt : kt + 1], None,
                                  ALU.is_equal)

        # counter-only coefficient work hoisted ahead of the ReduceScatter
        rt2 = rp.tile([CH, 1], F32, name="rt2")
        nc.vector.tensor_scalar(rt2[:], ctr_sb[:], 1.0, None, ALU.add)
        nc.vector.reciprocal(rt2[:], rt2[:])
        A2p = rp.tile([CH, 1], F32, name="A2p")
        nc.vector.tensor_tensor(out=A2p[:], in0=ctr_sb[:], in1=rt2[:], op=ALU.mult)
        nc.vector.tensor_scalar(A2p[:], A2p[:], 1.0, None, ALU.subtract)
        nc.vector.tensor_scalar(A2p[:], A2p[:], 2.0, None, ALU.mult)
        nc.vector.tensor_scalar(rt2[:], rt2[:], 2.0, None, ALU.mult)
        twos_c = rp.tile([CH, 1], F32, name="twosc")
        nc.vector.memset(twos_c[:], 2.0)


        # ---- phase 1: segment sums via DoubleRow fp8 ----
        # Two passes of 4 class-chunks (4x psA + 4x psB = 8 PSUM banks);
        # pass 1 is row-pair-major so it pipelines with one-hot generation.
        # ---- phase 1: segment sums via DoubleRow fp8 ----
        # Single pr-major pass over all 8 class chunks (8 PSUM banks), fully
        # pipelined with one-hot generation. The tiny counts matmuls run
        # per-chunk afterwards on each bank as its sums copy frees it.
        with tc.tile_pool(name="ps_seg", bufs=1, space="PSUM") as psg:
            psAs = {cc: psg.tile([CH, 512], F32, tag=f"psA{cc}",
                                 name=f"psA{cc}") for cc in range(8)}
            for pr in range(PR):
                for cc in range(8):
                    nc.tensor.matmul(
                        psAs[cc][:], ohv[:, pr, :, cc * CH : (cc + 1) * CH],
                        eav[:, pr, :, 0:512],
                        start=(pr == 0), stop=(pr == PR - 1), perf_mode=DRM,
                    )
            for cc in range(8):
                # copies on DVE (Act runs the e_sq squares; Pool must stay
                # clear so the ReduceScatter can start early)
                nc.vector.tensor_copy(out=ssv[:, cc, 0:512], in_=psAs[cc][:])
            for cc in range(8):
                # counts: reuse chunk cc's bank (same tag) once copied out
                psB = psg.tile([CH, 512], F32, tag=f"psA{cc}", name=f"psB{cc}")
                for pr in range(PR):
                    nc.tensor.matmul(
                        psB[:, 0:4], ohv[:, pr, :, cc * CH : (cc + 1) * CH],
                        eav[:, pr, :, 512:516],
                        start=(pr == 0), stop=(pr == PR - 1), perf_mode=DRM,
                    )
                nc.vector.tensor_copy(out=ssv[:, cc, 512:516], in_=psB[:, 0:4])
                # ccin DMAs ride the Pool queue (SP is busy with eb loads;
                # Pool is idle between one-hot gen and the ReduceScatter)
                nc.gpsimd.dma_start(ccin[cc * CH : (cc + 1) * CH, :],
                                    ssv[:, cc, :])

        # ---- ReduceScatter (sums+counts, bf16) ----
        nc.gpsimd.collective_compute(
            "ReduceScatter", ALU.add,
            replica_groups=[list(range(W))],
            ins=[ccin.opt()], outs=[rsout.opt()],
        )

        # ---- e_sq: eb loads on SP, Square+accumulate on Act (Act is
        # otherwise idle until phase 2; runs through the collectives) ----
        for pr in range(PR):
            ebt = ebp.tile([P, 2 * D], BF16, tag="eb", name="eb")
            nc.sync.dma_start(ebt[:], eb_ext[pr])
            eb_tiles.append(ebt)
            for j in (0, 1):
                kt = 2 * pr + j
                scr = sqp.tile([P, D], BF16, tag="scr", name="scr")
                nc.scalar.activation(
                    scr[:], ebt[:, j * D : (j + 1) * D], ACTF.Square,
                    accum_out=e_sq[:, kt : kt + 1],
                )
        # negate e_sq once (used as per-partition bias in phase 2)
        nc.scalar.mul(e_sq[:], e_sq[:], -1.0)

        # ---- embT load into ea_full's slot (overlaps the ReduceScatter) ----
        embT = bigp.tile([P, PR * 2 * FA], F8, tag="big", name="embT")
        etv = embT.rearrange("p (q n) -> p q n", q=4)[:, :, 0:NL]
        for q in range(4):
            nc.sync.dma_start(etv[:, q, :], et_ext[:, q * NL : (q + 1) * NL])

        # ---- per-rank prototype update (125 classes) ----
        # B2 = 2*rep*rm*rt ; A2 = 2*(1 + rep*(ctr*rt - 1)); rt2=2rt and
        # A2p=ctr*rt-1 were precomputed before the ReduceScatter.
        shard = rp.tile([CH, FU], BF16, name="shard")
        nc.sync.dma_start(shard[:], rsout[:])
        counts = shard[:, 512:513]
        rm = rp.tile([CH, 1], F32, name="rm")
        nc.vector.tensor_scalar(rm[:], counts, 1.0, None, ALU.max)
        nc.vector.reciprocal(rm[:], rm[:])
        rep = rp.tile([CH, 1], F32, name="rep")
        nc.vector.tensor_scalar(rep[:], counts, 0.0, None, ALU.is_gt)
        B2 = rp.tile([CH, 1], F32, name="B2")
        nc.vector.scalar_tensor_tensor(
            out=B2[:], in0=rm[:], scalar=rt2[:], in1=rep[:],
            op0=ALU.mult, op1=ALU.mult,
        )
        A2 = rp.tile([CH, 1], F32, name="A2")
        nc.vector.scalar_tensor_tensor(
            out=A2[:], in0=A2p[:], scalar=rep[:], in1=twos_c[:],
            op0=ALU.mult, op1=ALU.add,
        )

        tB = rp.tile([CH, D], F32, name="tB")
        nc.vector.tensor_scalar(tB[:], shard[:, 0:512], B2[:], None, ALU.mult)
        p2t_b = rp.tile([CH, D], BF16, name="p2tb")
        nc.vector.scalar_tensor_tensor(
            out=p2t_b[:], in0=p0s_sb[:], scalar=A2[:], in1=tB[:],
            op0=ALU.mult, op1=ALU.add,
        )
        # -p_sq = -0.25 * sum_d protos2^2
        scr2 = rp.tile([CH, D], BF16, name="scr2")
        npsq_b = rp.tile([CH, 1], BF16, name="npsqb")
        nc.vector.scalar_tensor_tensor(
            out=scr2[:], in0=p2t_b[:], scalar=-0.25, in1=p2t_b[:],
            op0=ALU.mult, op1=ALU.mult, accum_out=npsq_b[:],
        )

        # transpose this rank's protos2T to [d, c] BEFORE the AllGather so
        # no transpose work sits on the post-collective critical path
        agst = rp.tile([P, 4 * P], F8, name="agst")
        agsv = agst.rearrange("p (dc c) -> p dc c", dc=4)
        psq_st = rp.tile([1, P], F8, name="psqst")
        with tc.tile_pool(name="ps_tr", bufs=1, space="PSUM") as pst:
            t2 = pst.tile([P, 4 * P], BF16, tag="t2", name="t2")
            t2v = t2.rearrange("p (dc c) -> p dc c", dc=4)
            for dc in range(4):
                nc.tensor.matmul(
                    t2v[:, dc, :], p2t_b[:, dc * P : (dc + 1) * P], identb[:],
                    is_transpose=True, start=(dc == 0), stop=(dc == 3),
                )
            tq2 = pst.tile([1, P], BF16, tag="tq2", name="tq2")
            nc.tensor.matmul(tq2[:], npsq_b[:], identb[:],
                             is_transpose=True, start=True, stop=True)
            nc.vector.tensor_copy(out=agst[:], in_=t2[:])
            nc.vector.tensor_copy(out=psq_st[:], in_=tq2[:])
        # agin rows 0..511 = protos2 chunk [d, c]; row 512 = -p_sq row
        nc.sync.dma_start(
            agin[0:512, :].rearrange("(dc p) c -> p dc c", dc=4),
            agsv[:, :, :],
        )
        nc.sync.dma_start(agin[512:513, :], psq_st[:])

        # ---- AllGather (protos2T + -p_sq, fp8) ----
        nc.gpsimd.collective_compute(
            "AllGather", ALU.bypass,
            replica_groups=[list(range(W))],
            ins=[agin.opt()], outs=[agout.opt()],
        )

        # ---- load gathered protos2 (already [d, c] per rank) + -p_sq row ----
        agov = agout.rearrange("(r q) c -> q r c", r=W)
        for dc in range(4):
            eng = nc.sync if dc < 2 else nc.scalar
            eng.dma_start(
                p2sb.rearrange("p (dc r c) -> p dc r c", dc=4, r=W)[:, dc, :, :],
                agov[dc * P : (dc + 1) * P, :, :],
            )
        nc.sync.dma_start(
            psq2b[0:1, 0:CP].rearrange("a (r c) -> a r c", r=W),
            agov[512:513, :, :],
        )

        # ---- phase 2: out = 2*emb@protosT - e_sq - p_sq ----
        with tc.tile_pool(name="ps_cr", bufs=4, space="PSUM") as ps_cr:
            for nt in range(KT):
                ot = otp.tile([P, CP], F32, tag="ot", name="ot")
                for h in range(2):
                    c0 = 512 * h
                    cr = ps_cr.tile([P, 512], F32, tag=f"cr{h}", name=f"cr{h}")
                    nc.tensor.matmul(
                        cr[:],
                        ones2b.rearrange("k (j m) -> k j m", j=2)[:, :, :],
                        psq2b.rearrange("k (j c) -> k j c", j=2)[:, :, c0 : c0 + 512],
                        start=True, stop=False, perf_mode=DRM,
                    )
                    for q in range(2):
                        nc.tensor.matmul(
                            cr[:],
                            etv[:, 2 * q : 2 * q + 2, nt * P : (nt + 1) * P],
                            p2v[:, 2 * q : 2 * q + 2, c0 : c0 + 512],
                            start=False, stop=(q == 1), perf_mode=DRM,
                        )
                    # epilogue: add -e_sq while copying psum -> sbuf
                    # (gpsimd cannot access PSUM, so Act/DVE only)
                    if (2 * nt + h) % 9 in (0, 2, 4, 6):
                        nc.scalar.activation(
                            ot[:, c0 : c0 + 512], cr[:], ACTF.Identity,
                            bias=e_sq[:, nt : nt + 1], scale=1.0,
                        )
                    else:
                        nc.vector.tensor_scalar(
                            ot[:, c0 : c0 + 512], cr[:], e_sq[:, nt : nt + 1],
                            None, ALU.add,
                        )
                # output DMA rotation, finely interleaved: SP ~28, Pool ~28, Act ~8
                r = nt % 9
                if r in (0, 2, 4, 6):
                    eng = nc.sync
                elif r in (1, 3, 5, 7):
                    eng = nc.gpsimd
                else:
                    eng = nc.scalar
                eng.dma_start(out_ext[nt * P : (nt + 1) * P, :], ot[:, 0:C])

    _split_waits(nc)
    return nc


def _prep_inputs(embeddings, prototypes, counter, y_true):
    """Host-side sharding + layout prep (no kernel math beyond dtype casts)."""
    emb = np.ascontiguousarray(np.asarray(embeddings, dtype=np.float32))
    p0 = np.ascontiguousarray(np.asarray(prototypes, dtype=np.float32))
    ctr = np.ascontiguousarray(np.asarray(counter, dtype=np.float32))
    y = np.asarray(y_true)

    f8 = ml_dtypes.float8_e4m3
    bf = ml_dtypes.bfloat16

    p0_pad = np.zeros((CP, D), dtype=np.float32)
    p0_pad[0:C] = p0
    ctr_pad = np.zeros((CP,), dtype=np.float32)
    ctr_pad[0:C] = ctr

    in_maps = []
    for i in range(W):
        sl = slice(i * NL, (i + 1) * NL)
        e_i = emb[sl]                                   # [NL, D] f32
        # emb_aug fp8 pairs, partition-major: [PR, P, 2*FA]
        ea = np.zeros((NL, FA), dtype=f8)
        ea[:, 0:D] = e_i.astype(f8)
        ea[:, D] = 1.0
        ea_t = np.ascontiguousarray(
            ea.reshape(PR, 2, P, FA).transpose(0, 2, 1, 3).reshape(PR, P, 2 * FA)
        )
        # bf16 pairs for e_sq: [PR, P, 2*D]
        eb = e_i.astype(bf)
        eb_t = np.ascontiguousarray(
            eb.reshape(PR, 2, P, D).transpose(0, 2, 1, 3).reshape(PR, P, 2 * D)
        )
        # embT fp8: [P, 4*NL] with et[k, dc*NL + n] = emb[n, 128*dc + k]
        et = np.ascontiguousarray(
            e_i.astype(f8).T.reshape(4, P, NL).transpose(1, 0, 2).reshape(P, 4 * NL)
        )
        # labels, partition-major: yf[p, t] = y[t*128 + p]
        y_loc = y[sl].astype(np.float32)
        yf = np.ascontiguousarray(y_loc.reshape(KT, P).T)
        # per-rank class shard (class axis padded to CP)
        cs = slice(i * CH, (i + 1) * CH)
        in_maps.append(
            {
                "ea": ea_t,
                "eb": eb_t,
                "et": et,
                "yf": yf,
                "ctr": np.ascontiguousarray(ctr_pad[cs]).reshape(CH, 1),
                "p0s": np.ascontiguousarray(p0_pad[cs]),
            }
        )
    return in_maps


def kernel(embeddings, prototypes, counter, y_true):
    if _built[0] is None:
        _built[0] = _build()
    nc = _built[0]

    in_maps = _prep_inputs(embeddings, prototypes, counter, y_true)

    res = run_bass_kernel_spmd(
        nc, in_maps, list(range(W)), trace=PROFILE, **TRACE_KWARGS
    )
    LAST_RESULT[0] = res
    out = np.concatenate([res.results[i]["out"] for i in range(W)], axis=0)
    return out.astype(np.float32, copy=False)


# revision 46
# speedup vs baseline: 2.3533x; 1.0226x over previous
"""DeepNCM Trainium2 kernel: prototype scatter-mean update + negative squared
L2 distances, data-parallel over embedding rows across 8 NeuronCores.

Contract: kernel(**inputs) takes the FULL unsharded inputs
(embeddings [65536,512] f32, prototypes [1000,512] f32, counter [1000] f32,
y_true [65536] int64) and returns the FULL output [65536,1000] f32.

Per-core plan (NL = 8192 rows, fp8 DoubleRow matmuls throughout):
  Phase 1 (class-chunk major): sumsT[c, d] = oh^T @ emb_aug via DoubleRow fp8
     matmuls (two row-tiles contracted per instruction); emb_aug carries a
     ones column so per-class counts fall out of the same matmuls.
     One-hot tiles generated on DVE+Pool; e_sq accumulated from a bf16 copy
     of emb via DVE/Pool scalar_tensor_tensor (fp32 accumulator).
  ReduceScatter [1000,516] bf16 -> each rank owns 125 classes (sums+counts).
  Per-rank prototype update -> protos2T fp8 [125,512] + (-p_sq) col, then
  AllGather fp8 [1000,516]; PE transposes give protos2 [d, c] + (-p_sq) row.
  Phase 2: out = 2*emb@protosT - e_sq - p_sq: PSUM accumulates a K=1
     DoubleRow matmul seeding -p_sq, then 2 DoubleRow fp8 matmuls
     (embT pairs x protos2); epilogue copy adds -e_sq (per-partition bias)
     split across Act/DVE/Pool; output DMAs split across SP/Act/Pool queues.
"""

import os
import sys
from contextlib import ExitStack

for _p in ("/opt/trn_rl_repo", "/root/.axon_site/_ro/trn_rl_repo"):
    if os.path.isdir(_p):
        if _p not in sys.path:
            sys.path.insert(0, _p)
        break

import numpy as np
import ml_dtypes

import concourse.bass as bass
import concourse.mybir as mybir
import concourse.tile as tile
from concourse.masks import make_identity
from concourse.bass_utils import run_bass_kernel_spmd

N, D, C = 65536, 512, 1000
W = 8                      # cores
NL = N // W                # 8192 rows per core
P = 128
KT = NL // P               # 64 row tiles per core
PR = KT // 2               # 32 row-tile pairs (DoubleRow)
CP = 1024                  # class axis padded to 8 chunks of 128
CH = 128                   # classes per rank / class chunk (incl. padding)
FA = 520                   # emb_aug width: 512 emb + ones col + 7 pad
FU = 516                   # used width in collective buffers
F32 = mybir.dt.float32
BF16 = mybir.dt.bfloat16
F8 = mybir.dt.float8e4
ALU = mybir.AluOpType
ACTF = mybir.ActivationFunctionType
DRM = mybir.MatmulPerfMode.DoubleRow

# Toggled by test.py for profiling runs.
PROFILE = False
TRACE_KWARGS = {}
LAST_RESULT = [None]

_built = [None]


def _split_waits(nc, cap=1):
    """Walrus in this container rejects >1 sync-wait per instruction.
    Move excess waits onto preceding same-engine NOPs (in-order engines,
    so semantics are preserved)."""
    n_new = 0
    for fn in nc.m.functions:
        for bb in fn.blocks:
            new_list = []
            for ins in bb.instructions:
                si = getattr(ins, "sync_info", None)
                if si is not None and si.on_wait and len(si.on_wait) > cap:
                    waits = list(si.on_wait)
                    keep, rest = waits[:cap], waits[cap:]
                    for i in range(0, len(rest), cap):
                        nop = mybir.InstNoOp(
                            name=f"I-waitsplit-{n_new}", ins=[], outs=[]
                        )
                        n_new += 1
                        nop.engine = ins.engine
                        nop.sync_info = mybir.SyncInfo(
                            on_wait=rest[i : i + cap], on_update=[]
                        )
                        new_list.append(nop)
                    si.on_wait = keep
                new_list.append(ins)
            bb.instructions = new_list
    return n_new


def _build():
    nc = bass.Bass()
    ea_ext = nc.declare_dram_parameter("ea", [PR, P, 2 * FA], F8, isOutput=False)
    eb_ext = nc.declare_dram_parameter("eb", [PR, P, 2 * D], BF16, isOutput=False)
    et_ext = nc.declare_dram_parameter("et", [P, 4 * NL], F8, isOutput=False)
    yf_ext = nc.declare_dram_parameter("yf", [P, KT], F32, isOutput=False)
    ctr_ext = nc.declare_dram_parameter("ctr", [CH, 1], F32, isOutput=False)
    p0s_ext = nc.declare_dram_parameter("p0s", [CH, D], F32, isOutput=False)
    out_ext = nc.declare_dram_parameter("out", [NL, C], F32, isOutput=True)

    with tile.TileContext(nc) as tc, ExitStack() as es:
        cpool = es.enter_context(tc.tile_pool(name="const", bufs=1))
        bpool = es.enter_context(tc.tile_pool(name="bigs", bufs=1))
        bigp = es.enter_context(tc.tile_pool(name="bigp", bufs=1))
        ebp = es.enter_context(tc.tile_pool(name="ebp", bufs=20))
        sqp = es.enter_context(tc.tile_pool(name="sqp", bufs=2))
        rp = es.enter_context(tc.tile_pool(name="rp", bufs=1))
        otp = es.enter_context(tc.tile_pool(name="otp", bufs=6))
        dram = es.enter_context(tc.tile_pool(name="dram", bufs=1, space="DRAM"))

        # ---- constants ----
        iota = cpool.tile([P, CP], F32, name="iota")
        nc.gpsimd.iota(
            iota[:], pattern=[[1, CP]], base=0, channel_multiplier=0,
            allow_small_or_imprecise_dtypes=True,
        )
        identb = cpool.tile([P, P], BF16, name="identb")
        make_identity(nc, identb[:])
        ones2b = cpool.tile([2, 2 * P], F8, name="ones2b")
        nc.vector.memset(ones2b[:], 0.0)
        nc.vector.memset(ones2b[0:1, 0:P], 1.0)

        y_sb = cpool.tile([P, KT], F32, name="y")
        nc.sync.dma_start(y_sb[:], yf_ext[:])
        ctr_sb = rp.tile([CH, 1], F32, name="ctr")
        nc.sync.dma_start(ctr_sb[:], ctr_ext[:])
        p0s_sb = cpool.tile([CH, D], F32, name="p0s")
        nc.sync.dma_start(p0s_sb[:], p0s_ext[:])
        e_sq = cpool.tile([P, KT], F32, name="esq")

        # ---- big resident buffers ----
        # ea_full (phase 1) and embT (phase 2) are never live at the same
        # time: share one pool slot (same tag) to free 32KB/partition.
        ea_full = bigp.tile([P, PR * 2 * FA], F8, tag="big", name="ea")
        eav = ea_full.rearrange("p (pr j f) -> p pr j f", pr=PR, j=2)
        oh_full = bpool.tile([P, KT * CP], F8, name="oh")
        ohv = oh_full.rearrange("p (pr j c) -> p pr j c", pr=PR, j=2)
        p2sb = bpool.tile([P, 4 * CP], F8, name="p2sb")
        p2v = p2sb.rearrange("p (dc c) -> p dc c", dc=4)  # c = CP
        psq2b = cpool.tile([2, 2 * CP], F8, name="psq2b")
        nc.vector.memset(psq2b[:], 0.0)
        ss = bpool.tile([P, 8 * FU], BF16, name="ss")
        ssv = ss.rearrange("p (cc f) -> p cc f", cc=8)

        # collective DRAM buffers
        ccin = dram.tile([CP, FU], BF16, name="ccin")
        rsout = dram.tile([CH, FU], BF16, name="rsout")
        agin = dram.tile([513, P], F8, name="agin")
        agout = dram.tile([W * 513, P], F8, name="agout", addr_space="Shared")

        # ---- phase 1: loads + one-hot ----
        eb_tiles = []
        for pr in range(PR):
            nc.sync.dma_start(
                eav[:, pr, :, :], ea_ext[pr]
            )
            for j in (0, 1):
                kt = 2 * pr + j
                dst = ohv[:, pr, j, :]
                # split one-hot generation DVE : Pool roughly 39:25
                eng = nc.vector if (kt % 16) < 10 else nc.gpsimd
                eng.tensor_scalar(dst, iota[:], y_sb[:, kt : kt + 1], None,
                                  ALU.is_equal)

        # counter-only coefficient work hoisted ahead of the ReduceScatter
        rt2 = rp.tile([CH, 1], F32, name="rt2")
        nc.vector.tensor_scalar(rt2[:], ctr_sb[:], 1.0, None, ALU.add)
        nc.vector.reciprocal(rt2[:], rt2[:])
        A2p = rp.tile([CH, 1], F32, name="A2p")
        nc.vector.tensor_tensor(out=A2p[:], in0=ctr_sb[:], in1=rt2[:], op=ALU.mult)
        nc.vector.tensor_scalar(A2p[:], A2p[:], 1.0, None, ALU.subtract)
        nc.vector.tensor_scalar(A2p[:], A2p[:], 2.0, None, ALU.mult)
        nc.vector.tensor_scalar(rt2[:], rt2[:], 2.0, None, ALU.mult)
        twos_c = rp.tile([CH, 1], F32, name="twosc")
        nc.vector.memset(twos_c[:], 2.0)


        # ---- phase 1: segment sums via DoubleRow fp8 ----
        # Two passes of 4 class-chunks (4x psA + 4x psB = 8 PSUM banks);
        # pass 1 is row-pair-major so it pipelines with one-hot generation.
        # ---- phase 1: segment sums via DoubleRow fp8 ----
        # Single pr-major pass over all 8 class chunks (8 PSUM banks), fully
        # pipelined with one-hot generation. The tiny counts matmuls run
        # per-chunk afterwards on each bank as its sums copy frees it.
        with tc.tile_pool(name="ps_seg", bufs=1, space="PSUM") as psg:
            psAs = {cc: psg.tile([CH, 512], F32, tag=f"psA{cc}",
                                 name=f"psA{cc}") for cc in range(8)}
            for pr in range(PR):
                for cc in range(8):
                    nc.tensor.matmul(
                        psAs[cc][:], ohv[:, pr, :, cc * CH : (cc + 1) * CH],
                        eav[:, pr, :, 0:512],
                        start=(pr == 0), stop=(pr == PR - 1), perf_mode=DRM,
                    )
            for cc in range(8):
                # copies on DVE (Act runs the e_sq squares; Pool must stay
                # clear so the ReduceScatter can start early)
                nc.vector.tensor_copy(out=ssv[:, cc, 0:512], in_=psAs[cc][:])
            for cc in range(8):
                # counts: reuse chunk cc's bank (same tag) once copied out
                psB = psg.tile([CH, 512], F32, tag=f"psA{cc}", name=f"psB{cc}")
                for pr in range(PR):
                    nc.tensor.matmul(
                        psB[:, 0:4], ohv[:, pr, :, cc * CH : (cc + 1) * CH],
                        eav[:, pr, :, 512:516],
                        start=(pr == 0), stop=(pr == PR - 1), perf_mode=DRM,
                    )
                nc.vector.tensor_copy(out=ssv[:, cc, 512:516], in_=psB[:, 0:4])
                # ccin DMAs ride the Pool queue (SP is busy with eb loads;
                # Pool is idle between one-hot gen and the ReduceScatter)
                nc.gpsimd.dma_start(ccin[cc * CH : (cc + 1) * CH, :],
                                    ssv[:, cc, :])

        # ---- ReduceScatter (sums+counts, bf16) ----
        nc.gpsimd.collective_compute(
            "ReduceScatter", ALU.add,
            replica_groups=[list(range(W))],
            ins=[ccin.opt()], outs=[rsout.opt()],
        )

        # ---- e_sq: eb loads on SP, Square+accumulate on Act (Act is
        # otherwise idle until phase 2; runs through the collectives) ----
        for pr in range(PR):
            ebt = ebp.tile([P, 2 * D], BF16, tag="eb", name="eb")
            nc.sync.dma_start(ebt[:], eb_ext[pr])
            eb_tiles.append(ebt)
            for j in (0, 1):
                kt = 2 * pr + j
                scr = sqp.tile([P, D], BF16, tag="scr", name="scr")
                nc.scalar.activation(
                    scr[:], ebt[:, j * D : (j + 1) * D], ACTF.Square,
                    accum_out=e_sq[:, kt : kt + 1],
                )
        # negate e_sq once (used as per-partition bias in phase 2)
        nc.scalar.mul(e_sq[:], e_sq[:], -1.0)

        # ---- embT load into ea_full's slot (overlaps the ReduceScatter) ----
        embT = bigp.tile([P, PR * 2 * FA], F8, tag="big", name="embT")
        etv = embT.rearrange("p (q n) -> p q n", q=4)[:, :, 0:NL]
        for q in range(4):
            nc.sync.dma_start(etv[:, q, :], et_ext[:, q * NL : (q + 1) * NL])

        # ---- per-rank prototype update (125 classes) ----
        # B2 = 2*rep*rm*rt ; A2 = 2*(1 + rep*(ctr*rt - 1)); rt2=2rt and
        # A2p=ctr*rt-1 were precomputed before the ReduceScatter.
        shard = rp.tile([CH, FU], BF16, name="shard")
        nc.sync.dma_start(shard[:], rsout[:])
        counts = shard[:, 512:513]
        rm = rp.tile([CH, 1], F32, name="rm")
        nc.vector.tensor_scalar(rm[:], counts, 1.0, None, ALU.max)
        nc.vector.reciprocal(rm[:], rm[:])
        rep = rp.tile([CH, 1], F32, name="rep")
        nc.vector.tensor_scalar(rep[:], counts, 0.0, None, ALU.is_gt)
        B2 = rp.tile([CH, 1], F32, name="B2")
        nc.vector.scalar_tensor_tensor(
            out=B2[:], in0=rm[:], scalar=rt2[:], in1=rep[:],
            op0=ALU.mult, op1=ALU.mult,
        )
        A2 = rp.tile([CH, 1], F32, name="A2")
        nc.vector.scalar_tensor_tensor(
            out=A2[:], in0=A2p[:], scalar=rep[:], in1=twos_c[:],
            op0=ALU.mult, op1=ALU.add,
        )

        tB = rp.tile([CH, D], F32, name="tB")
        nc.vector.tensor_scalar(tB[:], shard[:, 0:512], B2[:], None, ALU.mult)
        p2t_b = rp.tile([CH, D], BF16, name="p2tb")
        nc.vector.scalar_tensor_tensor(
            out=p2t_b[:], in0=p0s_sb[:], scalar=A2[:], in1=tB[:],
            op0=ALU.mult, op1=ALU.add,
        )
        # -p_sq = -0.25 * sum_d protos2^2
        scr2 = rp.tile([CH, D], BF16, name="scr2")
        npsq_b = rp.tile([CH, 1], BF16, name="npsqb")
        nc.vector.scalar_tensor_tensor(
            out=scr2[:], in0=p2t_b[:], scalar=-0.25, in1=p2t_b[:],
            op0=ALU.mult, op1=ALU.mult, accum_out=npsq_b[:],
        )

        # transpose this rank's protos2T to [d, c] BEFORE the AllGather so
        # no transpose work sits on the post-collective critical path
        agst = rp.tile([P, 4 * P], F8, name="agst")
        agsv = agst.rearrange("p (dc c) -> p dc c", dc=4)
        psq_st = rp.tile([1, P], F8, name="psqst")
        with tc.tile_pool(name="ps_tr", bufs=1, space="PSUM") as pst:
            t2 = pst.tile([P, 4 * P], BF16, tag="t2", name="t2")
            t2v = t2.rearrange("p (dc c) -> p dc c", dc=4)
            for dc in range(4):
                nc.tensor.matmul(
                    t2v[:, dc, :], p2t_b[:, dc * P : (dc + 1) * P], identb[:],
                    is_transpose=True, start=(dc == 0), stop=(dc == 3),
                )
            tq2 = pst.tile([1, P], BF16, tag="tq2", name="tq2")
            nc.tensor.matmul(tq2[:], npsq_b[:], identb[:],
                             is_transpose=True, start=True, stop=True)
            nc.vector.tensor_copy(out=agst[:], in_=t2[:])
            nc.vector.tensor_copy(out=psq_st[:], in_=tq2[:])
        # agin rows 0..511 = protos2 chunk [d, c]; row 512 = -p_sq row
        nc.sync.dma_start(
            agin[0:512, :].rearrange("(dc p) c -> p dc c", dc=4),
            agsv[:, :, :],
        )
        nc.sync.dma_start(agin[512:513, :], psq_st[:])

        # ---- AllGather (protos2T + -p_sq, fp8) ----
        nc.gpsimd.collective_compute(
            "AllGather", ALU.bypass,
            replica_groups=[list(range(W))],
            ins=[agin.opt()], outs=[agout.opt()],
        )

        # ---- load gathered protos2 (already [d, c] per rank) + -p_sq row ----
        agov = agout.rearrange("(r q) c -> q r c", r=W)
        for dc in range(4):
            eng = nc.sync if dc < 2 else nc.scalar
            eng.dma_start(
                p2sb.rearrange("p (dc r c) -> p dc r c", dc=4, r=W)[:, dc, :, :],
                agov[dc * P : (dc + 1) * P, :, :],
            )
        nc.sync.dma_start(
            psq2b[0:1, 0:CP].rearrange("a (r c) -> a r c", r=W),
            agov[512:513, :, :],
        )

        # ---- phase 2: out = 2*emb@protosT - e_sq - p_sq ----
        with tc.tile_pool(name="ps_cr", bufs=3, space="PSUM") as ps_cr:
            for nt in range(KT):
                ot = otp.tile([P, CP], F32, tag="ot", name="ot")
                cr = ps_cr.tile([P, CP], F32, tag="cr", name="cr")
                for h in range(2):
                    c0 = 512 * h
                    crh = cr[:, c0 : c0 + 512]
                    nc.tensor.matmul(
                        crh,
                        ones2b.rearrange("k (j m) -> k j m", j=2)[:, :, :],
                        psq2b.rearrange("k (j c) -> k j c", j=2)[:, :, c0 : c0 + 512],
                        start=True, stop=False, perf_mode=DRM,
                    )
                    for q in range(2):
                        nc.tensor.matmul(
                            crh,
                            etv[:, 2 * q : 2 * q + 2, nt * P : (nt + 1) * P],
                            p2v[:, 2 * q : 2 * q + 2, c0 : c0 + 512],
                            start=False, stop=(q == 1), perf_mode=DRM,
                        )
                # epilogue: one op per tile adds -e_sq while copying psum ->
                # sbuf (gpsimd cannot access PSUM, so Act/DVE only)
                if nt % 2 == 0:
                    nc.scalar.activation(
                        ot[:], cr[:], ACTF.Identity,
                        bias=e_sq[:, nt : nt + 1], scale=1.0,
                    )
                else:
                    nc.vector.tensor_scalar(
                        ot[:], cr[:], e_sq[:, nt : nt + 1], None, ALU.add,
                    )
                # output DMA rotation, finely interleaved: SP ~28, Pool ~28, Act ~8
                r = nt % 9
                if r in (0, 2, 4, 6):
                    eng = nc.sync
                elif r in (1, 3, 5, 7):
                    eng = nc.gpsimd
                else:
                    eng = nc.scalar
                eng.dma_start(out_ext[nt * P : (nt + 1) * P, :], ot[:, 0:C])

    _split_waits(nc)
    return nc


def _prep_inputs(embeddings, prototypes, counter, y_true):
    """Host-side sharding + layout prep (no kernel math beyond dtype casts)."""
    emb = np.ascontiguousarray(np.asarray(embeddings, dtype=np.float32))
    p0 = np.ascontiguousarray(np.asarray(prototypes, dtype=np.float32))
    ctr = np.ascontiguousarray(np.asarray(counter, dtype=np.float32))
    y = np.asarray(y_true)

    f8 = ml_dtypes.float8_e4m3
    bf = ml_dtypes.bfloat16

    p0_pad = np.zeros((CP, D), dtype=np.float32)
    p0_pad[0:C] = p0
    ctr_pad = np.zeros((CP,), dtype=np.float32)
    ctr_pad[0:C] = ctr

    in_maps = []
    for i in range(W):
        sl = slice(i * NL, (i + 1) * NL)
        e_i = emb[sl]                                   # [NL, D] f32
        # emb_aug fp8 pairs, partition-major: [PR, P, 2*FA]
        ea = np.zeros((NL, FA), dtype=f8)
        ea[:, 0:D] = e_i.astype(f8)
        ea[:, D] = 1.0
        ea_t = np.ascontiguousarray(
            ea.reshape(PR, 2, P, FA).transpose(0, 2, 1, 3).reshape(PR, P, 2 * FA)
        )
        # bf16 pairs for e_sq: [PR, P, 2*D]
        eb = e_i.astype(bf)
        eb_t = np.ascontiguousarray(
            eb.reshape(PR, 2, P, D).transpose(0, 2, 1, 3).reshape(PR, P, 2 * D)
        )
        # embT fp8: [P, 4*NL] with et[k, dc*NL + n] = emb[n, 128*dc + k]
        et = np.ascontiguousarray(
            e_i.astype(f8).T.reshape(4, P, NL).transpose(1, 0, 2).reshape(P, 4 * NL)
        )
        # labels, partition-major: yf[p, t] = y[t*128 + p]
        y_loc = y[sl].astype(np.float32)
        yf = np.ascontiguousarray(y_loc.reshape(KT, P).T)
        # per-rank class shard (class axis padded to CP)
        cs = slice(i * CH, (i + 1) * CH)
        in_maps.append(
            {
                "ea": ea_t,
                "eb": eb_t,
                "et": et,
                "yf": yf,
                "ctr": np.ascontiguousarray(ctr_pad[cs]).reshape(CH, 1),
                "p0s": np.ascontiguousarray(p0_pad[cs]),
            }
        )
    return in_maps


def kernel(embeddings, prototypes, counter, y_true):
    if _built[0] is None:
        _built[0] = _build()
    nc = _built[0]

    in_maps = _prep_inputs(embeddings, prototypes, counter, y_true)

    res = run_bass_kernel_spmd(
        nc, in_maps, list(range(W)), trace=PROFILE, **TRACE_KWARGS
    )
    LAST_RESULT[0] = res
    out = np.concatenate([res.results[i]["out"] for i in range(W)], axis=0)
    return out.astype(np.float32, copy=False)


# revision 53
# speedup vs baseline: 2.4165x; 1.0269x over previous
"""DeepNCM Trainium2 kernel: prototype scatter-mean update + negative squared
L2 distances, data-parallel over embedding rows across 8 NeuronCores.

Contract: kernel(**inputs) takes the FULL unsharded inputs
(embeddings [65536,512] f32, prototypes [1000,512] f32, counter [1000] f32,
y_true [65536] int64) and returns the FULL output [65536,1000] f32.

Per-core plan (NL = 8192 rows, fp8 DoubleRow matmuls throughout):
  Phase 1 (class-chunk major): sumsT[c, d] = oh^T @ emb_aug via DoubleRow fp8
     matmuls (two row-tiles contracted per instruction); emb_aug carries a
     ones column so per-class counts fall out of the same matmuls.
     One-hot tiles generated on DVE+Pool; e_sq accumulated from a bf16 copy
     of emb via DVE/Pool scalar_tensor_tensor (fp32 accumulator).
  ReduceScatter [1000,516] bf16 -> each rank owns 125 classes (sums+counts).
  Per-rank prototype update -> protos2T fp8 [125,512] + (-p_sq) col, then
  AllGather fp8 [1000,516]; PE transposes give protos2 [d, c] + (-p_sq) row.
  Phase 2: out = 2*emb@protosT - e_sq - p_sq: PSUM accumulates a K=1
     DoubleRow matmul seeding -p_sq, then 2 DoubleRow fp8 matmuls
     (embT pairs x protos2); epilogue copy adds -e_sq (per-partition bias)
     split across Act/DVE/Pool; output DMAs split across SP/Act/Pool queues.
"""

import os
import sys
from contextlib import ExitStack

for _p in ("/opt/trn_rl_repo", "/root/.axon_site/_ro/trn_rl_repo"):
    if os.path.isdir(_p):
        if _p not in sys.path:
            sys.path.insert(0, _p)
        break

import numpy as np
import ml_dtypes

import concourse.bass as bass
import concourse.mybir as mybir
import concourse.tile as tile
from concourse.masks import make_identity
from concourse.bass_utils import run_bass_kernel_spmd

N, D, C = 65536, 512, 1000
W = 8                      # cores
NL = N // W                # 8192 rows per core
P = 128
KT = NL // P               # 64 row tiles per core
PR = KT // 2               # 32 row-tile pairs (DoubleRow)
CP = 1024                  # class axis padded to 8 chunks of 128
CH = 128                   # classes per rank / class chunk (incl. padding)
FA = 520                   # emb_aug width: 512 emb + ones col + 7 pad
FU = 516                   # used width in collective buffers
F32 = mybir.dt.float32
BF16 = mybir.dt.bfloat16
F8 = mybir.dt.float8e4
ALU = mybir.AluOpType
ACTF = mybir.ActivationFunctionType
DRM = mybir.MatmulPerfMode.DoubleRow

# Toggled by test.py for profiling runs.
PROFILE = False
TRACE_KWARGS = {}
LAST_RESULT = [None]

_built = [None]


def _split_waits(nc, cap=1):
    """Walrus in this container rejects >1 sync-wait per instruction.
    Move excess waits onto preceding same-engine NOPs (in-order engines,
    so semantics are preserved)."""
    n_new = 0
    for fn in nc.m.functions:
        for bb in fn.blocks:
            new_list = []
            for ins in bb.instructions:
                si = getattr(ins, "sync_info", None)
                if si is not None and si.on_wait and len(si.on_wait) > cap:
                    waits = list(si.on_wait)
                    keep, rest = waits[:cap], waits[cap:]
                    for i in range(0, len(rest), cap):
                        nop = mybir.InstNoOp(
                            name=f"I-waitsplit-{n_new}", ins=[], outs=[]
                        )
                        n_new += 1
                        nop.engine = ins.engine
                        nop.sync_info = mybir.SyncInfo(
                            on_wait=rest[i : i + cap], on_update=[]
                        )
                        new_list.append(nop)
                    si.on_wait = keep
                new_list.append(ins)
            bb.instructions = new_list
    return n_new


def _build():
    nc = bass.Bass()
    ea_ext = nc.declare_dram_parameter("ea", [PR, P, 2 * FA], F8, isOutput=False)
    eb_ext = nc.declare_dram_parameter("eb", [PR, P, 2 * D], BF16, isOutput=False)
    et_ext = nc.declare_dram_parameter("et", [P, 4 * NL], F8, isOutput=False)
    yf_ext = nc.declare_dram_parameter("yf", [P, KT], F32, isOutput=False)
    ctr_ext = nc.declare_dram_parameter("ctr", [CH, 1], F32, isOutput=False)
    p0s_ext = nc.declare_dram_parameter("p0s", [CH, D], F32, isOutput=False)
    out_ext = nc.declare_dram_parameter("out", [NL, C], F32, isOutput=True)

    with tile.TileContext(nc) as tc, ExitStack() as es:
        cpool = es.enter_context(tc.tile_pool(name="const", bufs=1))
        bpool = es.enter_context(tc.tile_pool(name="bigs", bufs=1))
        bigp = es.enter_context(tc.tile_pool(name="bigp", bufs=1))
        ebp = es.enter_context(tc.tile_pool(name="ebp", bufs=20))
        sqp = es.enter_context(tc.tile_pool(name="sqp", bufs=2))
        rp = es.enter_context(tc.tile_pool(name="rp", bufs=1))
        otp = es.enter_context(tc.tile_pool(name="otp", bufs=8))
        dram = es.enter_context(tc.tile_pool(name="dram", bufs=1, space="DRAM"))

        # ---- constants ----
        iota = cpool.tile([P, CP], F32, name="iota")
        nc.gpsimd.iota(
            iota[:], pattern=[[1, CP]], base=0, channel_multiplier=0,
            allow_small_or_imprecise_dtypes=True,
        )
        identb = cpool.tile([P, P], BF16, name="identb")
        make_identity(nc, identb[:])
        ones2b = cpool.tile([2, 2 * P], F8, name="ones2b")
        nc.vector.memset(ones2b[:], 0.0)
        nc.vector.memset(ones2b[0:1, 0:P], 1.0)

        y_sb = cpool.tile([P, KT], F32, name="y")
        nc.sync.dma_start(y_sb[:], yf_ext[:])
        ctr_sb = rp.tile([CH, 1], F32, name="ctr")
        p0s_sb = cpool.tile([CH, D], F32, name="p0s")
        e_sq = cpool.tile([P, KT], F32, name="esq")

        # ---- big resident buffers ----
        # ea_full (phase 1) and embT (phase 2) are never live at the same
        # time: share one pool slot (same tag) to free 32KB/partition.
        ea_full = bigp.tile([P, PR * 2 * FA], F8, tag="big", name="ea")
        eav = ea_full.rearrange("p (pr j f) -> p pr j f", pr=PR, j=2)
        oh_full = bpool.tile([P, KT * CP], F8, name="oh")
        ohv = oh_full.rearrange("p (pr j c) -> p pr j c", pr=PR, j=2)
        p2sb = bpool.tile([P, 4 * CP], F8, name="p2sb")
        p2v = p2sb.rearrange("p (dc c) -> p dc c", dc=4)  # c = CP
        psq2b = cpool.tile([2, 2 * CP], F8, name="psq2b")
        nc.vector.memset(psq2b[:], 0.0)
        ss = bpool.tile([P, 8 * FU], BF16, name="ss")
        ssv = ss.rearrange("p (cc f) -> p cc f", cc=8)

        # collective DRAM buffers
        ccin = dram.tile([CP, FU], BF16, name="ccin")
        rsout = dram.tile([CH, FU], BF16, name="rsout")
        agin = dram.tile([513, P], F8, name="agin")
        agout = dram.tile([W * 513, P], F8, name="agout", addr_space="Shared")

        # ---- phase 1: loads + one-hot ----
        eb_tiles = []
        for pr in range(PR):
            nc.sync.dma_start(
                eav[:, pr, :, :], ea_ext[pr]
            )
            for j in (0, 1):
                kt = 2 * pr + j
                dst = ohv[:, pr, j, :]
                # split one-hot generation DVE : Pool roughly 39:25
                eng = nc.vector if (kt % 16) < 10 else nc.gpsimd
                eng.tensor_scalar(dst, iota[:], y_sb[:, kt : kt + 1], None,
                                  ALU.is_equal)

        # counter / prototype shard loads (needed only at the seam)
        nc.sync.dma_start(ctr_sb[:], ctr_ext[:])
        nc.sync.dma_start(p0s_sb[:], p0s_ext[:])

        # counter-only coefficient work hoisted ahead of the ReduceScatter
        rt2 = rp.tile([CH, 1], F32, name="rt2")
        nc.vector.tensor_scalar(rt2[:], ctr_sb[:], 1.0, None, ALU.add)
        nc.vector.reciprocal(rt2[:], rt2[:])
        A2p = rp.tile([CH, 1], F32, name="A2p")
        nc.vector.tensor_tensor(out=A2p[:], in0=ctr_sb[:], in1=rt2[:], op=ALU.mult)
        nc.vector.tensor_scalar(A2p[:], A2p[:], 1.0, None, ALU.subtract)
        nc.vector.tensor_scalar(A2p[:], A2p[:], 2.0, None, ALU.mult)
        nc.vector.tensor_scalar(rt2[:], rt2[:], 2.0, None, ALU.mult)
        twos_c = rp.tile([CH, 1], F32, name="twosc")
        nc.vector.memset(twos_c[:], 2.0)


        # ---- phase 1: segment sums via DoubleRow fp8 ----
        # Two passes of 4 class-chunks (4x psA + 4x psB = 8 PSUM banks);
        # pass 1 is row-pair-major so it pipelines with one-hot generation.
        # ---- phase 1: segment sums via DoubleRow fp8 ----
        # Single pr-major pass over all 8 class chunks (8 PSUM banks), fully
        # pipelined with one-hot generation. The tiny counts matmuls run
        # per-chunk afterwards on each bank as its sums copy frees it.
        with tc.tile_pool(name="ps_seg", bufs=1, space="PSUM") as psg:
            psAs = {cc: psg.tile([CH, 512], F32, tag=f"psA{cc}",
                                 name=f"psA{cc}") for cc in range(8)}
            for pr in range(PR):
                for cc in range(8):
                    nc.tensor.matmul(
                        psAs[cc][:], ohv[:, pr, :, cc * CH : (cc + 1) * CH],
                        eav[:, pr, :, 0:512],
                        start=(pr == 0), stop=(pr == PR - 1), perf_mode=DRM,
                    )
            psBs = {}
            for cc in range(8):
                # sums copy on DVE (Act runs the e_sq squares; Pool must stay
                # clear so the ReduceScatter can start early)
                nc.vector.tensor_copy(out=ssv[:, cc, 0:512], in_=psAs[cc][:])
                # counts: reuse chunk cc's bank (same tag) once copied out
                psB = psg.tile([CH, 512], F32, tag=f"psA{cc}", name=f"psB{cc}")
                psBs[cc] = psB
                for pr in range(PR):
                    nc.tensor.matmul(
                        psB[:, 0:4], ohv[:, pr, :, cc * CH : (cc + 1) * CH],
                        eav[:, pr, :, 512:516],
                        start=(pr == 0), stop=(pr == PR - 1), perf_mode=DRM,
                    )
                nc.vector.tensor_copy(out=ssv[:, cc, 512:516], in_=psB[:, 0:4])
                # ccin DMAs ride the Pool queue (SP is busy with eb loads;
                # Pool is idle between one-hot gen and the ReduceScatter)
                nc.gpsimd.dma_start(ccin[cc * CH : (cc + 1) * CH, :],
                                    ssv[:, cc, :])

        # ---- ReduceScatter (sums+counts, bf16) ----
        nc.gpsimd.collective_compute(
            "ReduceScatter", ALU.add,
            replica_groups=[list(range(W))],
            ins=[ccin.opt()], outs=[rsout.opt()],
        )

        # ---- e_sq: eb loads on SP; squares on Act (pairs 0-19) and DVE
        # (pairs 20-31, running inside the ReduceScatter window) ----
        for pr in range(PR):
            ebt = ebp.tile([P, 2 * D], BF16, tag="eb", name="eb")
            nc.sync.dma_start(ebt[:], eb_ext[pr])
            eb_tiles.append(ebt)
            for j in (0, 1):
                kt = 2 * pr + j
                scr = sqp.tile([P, D], BF16, tag="scr", name="scr")
                nc.scalar.activation(
                    scr[:], ebt[:, j * D : (j + 1) * D], ACTF.Square,
                    accum_out=e_sq[:, kt : kt + 1],
                )

        # negate e_sq once (used as per-partition bias in phase 2)
        nc.scalar.mul(e_sq[:], e_sq[:], -1.0)

        # ---- embT load into ea_full's slot (overlaps the ReduceScatter) ----
        embT = bigp.tile([P, PR * 2 * FA], F8, tag="big", name="embT")
        etv = embT.rearrange("p (q n) -> p q n", q=4)[:, :, 0:NL]
        for q in range(4):
            nc.sync.dma_start(etv[:, q, :], et_ext[:, q * NL : (q + 1) * NL])

        # ---- per-rank prototype update (125 classes) ----
        # B2 = 2*rep*rm*rt ; A2 = 2*(1 + rep*(ctr*rt - 1)); rt2=2rt and
        # A2p=ctr*rt-1 were precomputed before the ReduceScatter.
        shard = rp.tile([CH, FU], BF16, name="shard")
        nc.sync.dma_start(shard[:], rsout[:])
        counts = shard[:, 512:513]
        rm = rp.tile([CH, 1], F32, name="rm")
        nc.vector.tensor_scalar(rm[:], counts, 1.0, None, ALU.max)
        nc.vector.reciprocal(rm[:], rm[:])
        rep = rp.tile([CH, 1], F32, name="rep")
        nc.vector.tensor_scalar(rep[:], counts, 0.0, None, ALU.is_gt)
        B2 = rp.tile([CH, 1], F32, name="B2")
        nc.vector.scalar_tensor_tensor(
            out=B2[:], in0=rm[:], scalar=rt2[:], in1=rep[:],
            op0=ALU.mult, op1=ALU.mult,
        )
        A2 = rp.tile([CH, 1], F32, name="A2")
        nc.vector.scalar_tensor_tensor(
            out=A2[:], in0=A2p[:], scalar=rep[:], in1=twos_c[:],
            op0=ALU.mult, op1=ALU.add,
        )

        tB = rp.tile([CH, D], F32, name="tB")
        nc.vector.tensor_scalar(tB[:], shard[:, 0:512], B2[:], None, ALU.mult)
        p2t_b = rp.tile([CH, D], BF16, name="p2tb")
        nc.vector.scalar_tensor_tensor(
            out=p2t_b[:], in0=p0s_sb[:], scalar=A2[:], in1=tB[:],
            op0=ALU.mult, op1=ALU.add,
        )
        # -p_sq = -0.25 * sum_d protos2^2
        scr2 = rp.tile([CH, D], BF16, name="scr2")
        npsq_b = rp.tile([CH, 1], BF16, name="npsqb")
        nc.vector.scalar_tensor_tensor(
            out=scr2[:], in0=p2t_b[:], scalar=-0.25, in1=p2t_b[:],
            op0=ALU.mult, op1=ALU.mult, accum_out=npsq_b[:],
        )

        # transpose this rank's protos2T to [d, c] BEFORE the AllGather so
        # no transpose work sits on the post-collective critical path
        agst = rp.tile([P, 4 * P], F8, name="agst")
        agsv = agst.rearrange("p (dc c) -> p dc c", dc=4)
        psq_st = rp.tile([1, P], F8, name="psqst")
        with tc.tile_pool(name="ps_tr", bufs=1, space="PSUM") as pst:
            t2 = pst.tile([P, 4 * P], BF16, tag="t2", name="t2")
            t2v = t2.rearrange("p (dc c) -> p dc c", dc=4)
            for dc in range(4):
                nc.tensor.matmul(
                    t2v[:, dc, :], p2t_b[:, dc * P : (dc + 1) * P], identb[:],
                    is_transpose=True, start=(dc == 0), stop=(dc == 3),
                )
            tq2 = pst.tile([1, P], BF16, tag="tq2", name="tq2")
            nc.tensor.matmul(tq2[:], npsq_b[:], identb[:],
                             is_transpose=True, start=True, stop=True)
            nc.vector.tensor_copy(out=agst[:], in_=t2[:])
            nc.vector.tensor_copy(out=psq_st[:], in_=tq2[:])
        # agin rows 0..511 = protos2 chunk [d, c]; row 512 = -p_sq row
        nc.sync.dma_start(
            agin[0:512, :].rearrange("(dc p) c -> p dc c", dc=4),
            agsv[:, :, :],
        )
        nc.sync.dma_start(agin[512:513, :], psq_st[:])

        # ---- AllGather (protos2T + -p_sq, fp8) ----
        nc.gpsimd.collective_compute(
            "AllGather", ALU.bypass,
            replica_groups=[list(range(W))],
            ins=[agin.opt()], outs=[agout.opt()],
        )

        # ---- load gathered protos2 (already [d, c] per rank) + -p_sq row ----
        # psq row first: the p_sq seed matmul opens every accumulation group
        agov = agout.rearrange("(r q) c -> q r c", r=W)
        nc.sync.dma_start(
            psq2b[0:1, 0:CP].rearrange("a (r c) -> a r c", r=W),
            agov[512:513, :, :],
        )
        for dc in range(4):
            eng = nc.scalar if dc < 2 else nc.sync
            eng.dma_start(
                p2sb.rearrange("p (dc r c) -> p dc r c", dc=4, r=W)[:, dc, :, :],
                agov[dc * P : (dc + 1) * P, :, :],
            )

        # ---- phase 2: out = 2*emb@protosT - e_sq - p_sq ----
        with tc.tile_pool(name="ps_cr", bufs=4, space="PSUM") as ps_cr:
            for nt in range(KT):
                ot = otp.tile([P, CP], F32, tag="ot", name="ot")
                for h in range(2):
                    c0 = 512 * h
                    cr = ps_cr.tile([P, 512], F32, tag=f"cr{h}", name=f"cr{h}")
                    nc.tensor.matmul(
                        cr[:],
                        ones2b.rearrange("k (j m) -> k j m", j=2)[:, :, :],
                        psq2b.rearrange("k (j c) -> k j c", j=2)[:, :, c0 : c0 + 512],
                        start=True, stop=False, perf_mode=DRM,
                    )
                    for q in range(2):
                        nc.tensor.matmul(
                            cr[:],
                            etv[:, 2 * q : 2 * q + 2, nt * P : (nt + 1) * P],
                            p2v[:, 2 * q : 2 * q + 2, c0 : c0 + 512],
                            start=False, stop=(q == 1), perf_mode=DRM,
                        )
                    # epilogue: add -e_sq while copying psum -> sbuf
                    # (gpsimd cannot access PSUM, so Act/DVE only)
                    if (2 * nt + h) % 9 in (0, 2, 4, 6):
                        nc.scalar.activation(
                            ot[:, c0 : c0 + 512], cr[:], ACTF.Identity,
                            bias=e_sq[:, nt : nt + 1], scale=1.0,
                        )
                    else:
                        nc.vector.tensor_scalar(
                            ot[:, c0 : c0 + 512], cr[:], e_sq[:, nt : nt + 1],
                            None, ALU.add,
                        )
                # output DMA rotation, finely interleaved: SP ~28, Pool ~28, Act ~8
                r = nt % 9
                if r in (0, 2, 4, 6):
                    eng = nc.sync
                elif r in (1, 3, 5, 7):
                    eng = nc.gpsimd
                else:
                    eng = nc.scalar
                eng.dma_start(out_ext[nt * P : (nt + 1) * P, :], ot[:, 0:C])

    _split_waits(nc)
    return nc


def _prep_inputs(embeddings, prototypes, counter, y_true):
    """Host-side sharding + layout prep (no kernel math beyond dtype casts)."""
    emb = np.ascontiguousarray(np.asarray(embeddings, dtype=np.float32))
    p0 = np.ascontiguousarray(np.asarray(prototypes, dtype=np.float32))
    ctr = np.ascontiguousarray(np.asarray(counter, dtype=np.float32))
    y = np.asarray(y_true)

    f8 = ml_dtypes.float8_e4m3
    bf = ml_dtypes.bfloat16

    p0_pad = np.zeros((CP, D), dtype=np.float32)
    p0_pad[0:C] = p0
    ctr_pad = np.zeros((CP,), dtype=np.float32)
    ctr_pad[0:C] = ctr

    in_maps = []
    for i in range(W):
        sl = slice(i * NL, (i + 1) * NL)
        e_i = emb[sl]                                   # [NL, D] f32
        # emb_aug fp8 pairs, partition-major: [PR, P, 2*FA]
        ea = np.zeros((NL, FA), dtype=f8)
        ea[:, 0:D] = e_i.astype(f8)
        ea[:, D] = 1.0
        ea_t = np.ascontiguousarray(
            ea.reshape(PR, 2, P, FA).transpose(0, 2, 1, 3).reshape(PR, P, 2 * FA)
        )
        # bf16 pairs for e_sq: [PR, P, 2*D]
        eb = e_i.astype(bf)
        eb_t = np.ascontiguousarray(
            eb.reshape(PR, 2, P, D).transpose(0, 2, 1, 3).reshape(PR, P, 2 * D)
        )
        # embT fp8: [P, 4*NL] with et[k, dc*NL + n] = emb[n, 128*dc + k]
        et = np.ascontiguousarray(
            e_i.astype(f8).T.reshape(4, P, NL).transpose(1, 0, 2).reshape(P, 4 * NL)
        )
        # labels, partition-major: yf[p, t] = y[t*128 + p]
        y_loc = y[sl].astype(np.float32)
        yf = np.ascontiguousarray(y_loc.reshape(KT, P).T)
        # per-rank class shard (class axis padded to CP)
        cs = slice(i * CH, (i + 1) * CH)
        in_maps.append(
            {
                "ea": ea_t,
                "eb": eb_t,
                "et": et,
                "yf": yf,
                "ctr": np.ascontiguousarray(ctr_pad[cs]).reshape(CH, 1),
                "p0s": np.ascontiguousarray(p0_pad[cs]),
            }
        )
    return in_maps


def kernel(embeddings, prototypes, counter, y_true):
    if _built[0] is None:
        _built[0] = _build()
    nc = _built[0]

    in_maps = _prep_inputs(embeddings, prototypes, counter, y_true)

    res = run_bass_kernel_spmd(
        nc, in_maps, list(range(W)), trace=PROFILE, **TRACE_KWARGS
    )
    LAST_RESULT[0] = res
    out = np.concatenate([res.results[i]["out"] for i in range(W)], axis=0)
    return out.astype(np.float32, copy=False)
